# revision 43
# baseline (speedup 1.0000x reference)
"""BiMamba (bidirectional Mamba2) Trainium2 kernel.

Sharding: 8 NeuronCores = 2 directions x 4 batch sequences; each core runs
the full Mamba2 block (LN -> in_proj -> conv -> chunked SSM scan -> gated
RMSNorm -> out_proj) for one (direction, batch) pair. Host does the
(cheap) sequence flip for the reverse direction and the final
average + LayerNorm combine.

v2: engine-rebalanced. Phase 4 builds the per-chunk decay matrices with
batched ops (one ACT exp per chunk instead of 32; DVE f16 2x-mode adds),
conv runs on DVE (tensor_scalar 4x mode) overlapped with the z GEMM on PE,
sz stays resident in SBUF, the SSM state is kept in f16 with the
decayed-state add done via an identity matmul into PSUM, norm_w is folded
into w_out on the host, and rstd is applied at out_proj eviction.
"""
import numpy as np
import concourse.bass as bass
import concourse.tile as tile
from concourse import bacc, mybir
from concourse import bass_utils
from concourse.masks import make_identity

F32 = mybir.dt.float32
F16 = mybir.dt.float16
I32 = mybir.dt.int32
AF = mybir.ActivationFunctionType
ALU = mybir.AluOpType
AX = mybir.AxisListType

L = 1024          # seq len
DM = 1024         # d_model
DI = 2048         # d_inner
H = 32            # nheads
PH = 64           # headdim
NS = 128          # d_state
CONV = 2304       # conv channels
EIN = 4384        # in_proj out dim
EPAD = 4480       # padded (35*128)
TC = 8            # time chunks
CH = 128          # chunk length
EPS = 1e-5
NEG = -30000.0


def _fast_rsqrt(nc, pool, out_ap, x_ap, magic_bcast, shape, tag):
    """out = 1/sqrt(x) via int bit-hack + 2 Newton iterations (DVE only).
    x_ap must be positive. shape = (128, n). magic_bcast: int32 AP broadcast
    of 0x5f3759df matching shape."""
    n = shape[1]
    sh = pool.tile([128, n], I32, tag=tag + "_sh")
    nc.vector.tensor_scalar(sh[:], x_ap.bitcast(I32), 1, None,
                            op0=ALU.logical_shift_right)
    y = pool.tile([128, n], F32, tag=tag + "_y")
    nc.vector.scalar_tensor_tensor(y[:].bitcast(I32), magic_bcast, 0,
                                   sh[:], op0=ALU.bypass, op1=ALU.subtract)
    xh = pool.tile([128, n], F32, tag=tag + "_xh")
    nc.vector.tensor_scalar_mul(xh[:], x_ap, 0.5)
    t = pool.tile([128, n], F32, tag=tag + "_t")
    for _ in range(2):
        nc.vector.tensor_tensor(t[:], y[:], y[:], op=ALU.mult)
        nc.vector.tensor_tensor(t[:], t[:], xh[:], op=ALU.mult)
        nc.vector.tensor_scalar(t[:], t[:], -1.0, 1.5, op0=ALU.mult, op1=ALU.add)
        nc.vector.tensor_tensor(y[:], y[:], t[:], op=ALU.mult)
    nc.vector.tensor_copy(out=out_ap, in_=y[:])


def _declare(nc):
    u_d = nc.dram_tensor("u", [L, DM], F32, kind="ExternalInput").ap()
    w_in_d = nc.dram_tensor("w_in", [DM, EPAD], F16, kind="ExternalInput").ap()
    w_out_d = nc.dram_tensor("w_out", [DI, DM], F16, kind="ExternalInput").ap()
    # packed small params: cols 0:72 conv_wt, 72:90 conv_bt,
    # 90 dt_bias (rows 0:32), 91 a_neg (rows 0:32)
    params_d = nc.dram_tensor("params", [128, 92], F32, kind="ExternalInput").ap()
    d_diag_d = nc.dram_tensor("d_diag", [H, 128, 128], F16, kind="ExternalInput").ap()
    e_ind_d = nc.dram_tensor("e_ind", [2 * H, H * CH], F16, kind="ExternalInput").ap()
    out_d = nc.dram_tensor("out", [L, DM], F32, kind="ExternalOutput").ap()
    return (u_d, w_in_d, w_out_d, params_d, d_diag_d, e_ind_d, out_d)


def _build(nc, repeats=1):
    args = _declare(nc)
    with tile.TileContext(nc) as tc:
        for _ in range(repeats):
            _body(nc, tc, *args, {})
    nc.compile()
    return nc


def _body(nc, tc, u_d, w_in_d, w_out_d, params_d, d_diag_d, e_ind_d, out_d, dbg_d):
    from contextlib import ExitStack
    ctx = ExitStack()
    with ctx:
        # ---------- constants / small params (whole-kernel lifetime) ----------
        const_p = ctx.enter_context(tc.tile_pool(name="const", bufs=1))
        ident16 = const_p.tile([128, 128], F16)
        make_identity(nc, ident16)
        ident32 = const_p.tile([128, 128], F32)
        make_identity(nc, ident32)
        maskB = const_p.tile([128, 128], F32)
        nc.gpsimd.memset(maskB[:], 0.0)
        # keep where t1 (free) >= t2 (partition): iota = free - part >= 0
        nc.gpsimd.affine_select(out=maskB[:], in_=maskB[:], pattern=[[1, 128]],
                                compare_op=ALU.is_ge, fill=NEG, base=0,
                                channel_multiplier=-1)
        eps_t = const_p.tile([128, 1], F32)
        nc.gpsimd.memset(eps_t[:], EPS)
        magic_t = const_p.tile([128, 1], I32)
        nc.gpsimd.memset(magic_t[:], 0x5F3759DF)
        params = const_p.tile([128, 92], F32)
        nc.sync.dma_start(params[:], params_d[:])
        conv_wt = params[:, 0:72]
        conv_bt = params[:, 72:90]
        dt_bias = params[0:32, 90:91]
        a_neg = params[0:32, 91:92]
        ones2 = const_p.tile([2, 128], F16)
        nc.gpsimd.memset(ones2[:], 1.0)

        # ---------- mid-size residents ----------
        res_p = ctx.enter_context(tc.tile_pool(name="res", bufs=1))
        BT_sb = res_p.tile([128, L], F16)         # [n, t]
        CT_sb = res_p.tile([128, L], F16)         # [n, t]
        B_t = res_p.tile([128, TC, NS], F16)      # [tp, tc, n]
        cnqr_sb = res_p.tile([2 + 2 * H, TC, CH], F16)  # [ones;colneg q/r] rows
        nc.gpsimd.memset(cnqr_sb[0:2, :, :], 1.0)
        ind_cas = []
        for par in range(2):
            t = res_p.tile([2 + 2 * H, H * CH], F16, name=f"indca{par}")
            ind_cas.append(t)
        nc.gpsimd.dma_start(ind_cas[0][2:, :], e_ind_d[:])
        nc.gpsimd.memset(ind_cas[0][0:2, :], 0.0)
        nc.vector.tensor_copy(out=ind_cas[1][:, :], in_=ind_cas[0][:, :])
        dtw_all = res_p.tile([128, TC, H], F16)   # dt * w per chunk
        gt_all = res_p.tile([128, TC, CH], F16)   # B^T C per chunk
        sz_sb = res_p.tile([128, TC, DI], F16)    # silu(z), resident  4 MB
        midp_cm = tc.tile_pool(name="midp", bufs=1)
        midp = midp_cm.__enter__()
        dt_sb = midp.tile([32, L], F32)           # [h, t]
        dt_T = midp.tile([128, TC, H], F16)       # [tp, tc, h]
        w_T = midp.tile([128, TC, H], F16)        # decay-to-chunk-end
        cA_row = midp.tile([32, TC, CH], F32)     # [h, tc, t]

        _uid = nc.next_id()
        caqr_dram = nc.dram_tensor(f"caqr_{_uid}", [TC, 2, H * CH], F16).ap()
        cn_dram = nc.dram_tensor(f"cn_{_uid}", [TC, 2 * H, CH], F16).ap()
        da0_dram = nc.dram_tensor(f"da0_bcast_{_uid}", [TC, H * CH], F16).ap()
        X_dram = nc.dram_tensor(f"x_spill_{_uid}", [TC, 128, DI], F16).ap()

        with tc.tile_pool(name="hTp", bufs=1) as hTp:
            hT = hTp.tile([128, 8, L], F16)        # [dp, do, t]   2 MB
            # ---------- phase 1: LN(u) -> h, transpose -> hT ----------
            with tc.tile_pool(name="ph1", bufs=2) as p1, \
                 tc.tile_pool(name="ph1u", bufs=8) as p1u, \
                 tc.tile_pool(name="ph1s", bufs=2) as p1s, \
                 tc.tile_pool(name="ph1ps", bufs=4, space="PSUM") as p1ps:
                dma_engs = [nc.sync, nc.gpsimd]
                for half in range(2):
                    u_ts = []
                    ssum_all = p1s.tile([128, 4], F32, tag="ssum")
                    ssq_all = p1s.tile([128, 4], F32, tag="ssq")
                    for j in range(4):
                        t_c = half * 4 + j
                        u_t = p1u.tile([128, DM], F32, tag="u", name=f"u{t_c}")
                        dma_engs[t_c % 2].dma_start(u_t[:], u_d[t_c * 128:(t_c + 1) * 128, :])
                        u_ts.append(u_t)
                        nc.vector.tensor_reduce(ssum_all[:, j:j + 1], u_t[:],
                                                axis=AX.X, op=ALU.add)
                        sq = p1.tile([128, DM], F32, tag="sq")
                        nc.scalar.activation(sq[:], u_t[:], AF.Square,
                                             accum_out=ssq_all[:, j:j + 1])
                    nmean = p1s.tile([128, 4], F32, tag="nmean")
                    nc.vector.tensor_scalar_mul(nmean[:], ssum_all[:], -1.0 / DM)
                    var = p1s.tile([128, 4], F32, tag="var")
                    nc.vector.tensor_tensor(var[:], nmean[:], nmean[:], op=ALU.mult)
                    nc.vector.scalar_tensor_tensor(var[:], ssq_all[:], 1.0 / DM, var[:],
                                                   op0=ALU.mult, op1=ALU.subtract)
                    nc.vector.tensor_scalar_add(var[:], var[:], EPS)
                    rstd = p1s.tile([128, 4], F32, tag="rstd")
                    _fast_rsqrt(nc, p1s, rstd[:], var[:],
                                magic_t[:].to_broadcast((128, 4)), (128, 4), "ln")
                    bias2 = p1s.tile([128, 4], F32, tag="bias2")
                    nc.vector.tensor_tensor(bias2[:], nmean[:], rstd[:], op=ALU.mult)
                    for j in range(4):
                        t_c = half * 4 + j
                        h_t = p1.tile([128, DM], F16, tag="h")
                        if t_c % 2 == 0:
                            nc.scalar.activation(h_t[:], u_ts[j][:], AF.Identity,
                                                 bias=bias2[:, j:j + 1],
                                                 scale=rstd[:, j:j + 1])
                        else:
                            nc.vector.tensor_scalar(h_t[:], u_ts[j][:],
                                                    rstd[:, j:j + 1],
                                                    bias2[:, j:j + 1],
                                                    op0=ALU.mult, op1=ALU.add)
                        for dq in range(2):
                            ps = p1ps.tile([128, 4, 128], F16, tag="trq")
                            for jj in range(4):
                                do = dq * 4 + jj
                                nc.tensor.transpose(ps[:, jj, :],
                                                    h_t[:, do * 128:(do + 1) * 128],
                                                    ident16[:])
                            nc.any.tensor_copy(
                                out=hT[:, dq * 4:(dq + 1) * 4, t_c * 128:(t_c + 1) * 128],
                                in_=ps[:])
            if "hT" in dbg_d:
                for a in range(8):
                    nc.sync.dma_start(dbg_d["hT"][a * 128:(a + 1) * 128, :], hT[:, a, :])

            # ---------- phase 2a: in_proj xBC/dt GEMM -> raw xr staging ----------
            with tc.tile_pool(name="p2xr", bufs=1) as xrp:
                xr_all = xrp.tile([128, 18, 1030], F16)    # 3 zero-pad + 1024 + align
                nc.gpsimd.memset(xr_all[:, :, 0:3], 0.0)
                with tc.tile_pool(name="p2w", bufs=4) as wp, \
                     tc.tile_pool(name="p2ps", bufs=2, space="PSUM") as pps:
                    for ec in range(19):
                        e0 = DI + ec * 128
                        m = 128 if ec < 18 else 32
                        ps = pps.tile([128, 2, 512], F32, tag="px")
                        wt = wp.tile([128, 8, 128], F16, tag="w")
                        nc.sync.dma_start(wt[:], w_in_d[:, e0:e0 + 128]
                                          .rearrange("(kd p) e -> p kd e", p=128))
                        for kd in range(8):
                            for th in range(2):
                                nc.tensor.matmul(ps[:m, th], lhsT=wt[:, kd, :m],
                                                 rhs=hT[:, kd, th * 512:(th + 1) * 512],
                                                 start=(kd == 0), stop=(kd == 7))
                        if ec == 18:
                            # softplus(x + dt_bias) = ln(1 + exp(x + dt_bias))
                            nc.scalar.activation(dt_sb[:], ps[:32].rearrange("p a b -> p (a b)"),
                                                 AF.Exp, bias=dt_bias[:])
                            nc.scalar.activation(dt_sb[:], dt_sb[:], AF.Ln, bias=1.0)
                        else:
                            nc.scalar.activation(xr_all[:, ec, 3:3 + L],
                                                 ps[:].rearrange("p a b -> p (a b)"),
                                                 AF.Copy)

                # ------- phase 2b + 3 interleaved: conv (DVE) + z GEMM (PE) -------
                # conv for one ec: 4 tensor_scalar (4x) + add tree -> silu -> out
                with tc.tile_pool(name="p3w", bufs=1) as wp3, \
                     tc.tile_pool(name="p3cv", bufs=3) as cvp, \
                     tc.tile_pool(name="p3ps", bufs=2, space="PSUM") as pz, \
                     tc.tile_pool(name="p3pt", bufs=2, space="PSUM") as ppt:

                    def emit_conv(ec):
                        xr = xr_all[:, ec, :]
                        ts = []
                        for k in range(4):
                            t = cvp.tile([128, L], F16, tag=f"cv{k}")
                            nc.vector.tensor_scalar(
                                t[:], xr[:, k:k + L], conv_wt[:, ec * 4 + k:ec * 4 + k + 1],
                                None, op0=ALU.mult)
                            ts.append(t)
                        nc.vector.tensor_tensor(ts[0][:], ts[0][:], ts[1][:], op=ALU.add)
                        nc.vector.tensor_tensor(ts[2][:], ts[2][:], ts[3][:], op=ALU.add)
                        nc.vector.tensor_tensor(ts[0][:], ts[0][:], ts[2][:], op=ALU.add)
                        if ec <= 15:
                            xa = cvp.tile([128, L], F16, tag="xact")
                            nc.scalar.activation(xa[:], ts[0][:], AF.Silu,
                                                 bias=conv_bt[:, ec:ec + 1])
                            tp_ps = ppt.tile([128, 8, 128], F16, tag="ptr")
                            for tcb in range(8):
                                nc.tensor.transpose(tp_ps[:, tcb, :],
                                                    xa[:, tcb * 128:(tcb + 1) * 128],
                                                    ident16[:])
                            xts = cvp.tile([128, 8, 128], F16, tag="xts")
                            nc.any.tensor_copy(out=xts[:], in_=tp_ps[:])
                            nc.sync.dma_start(
                                X_dram[:, :, ec * 128:(ec + 1) * 128]
                                .rearrange("a p c -> p a c"), xts[:])
                        elif ec == 16:
                            nc.scalar.activation(BT_sb[:], ts[0][:], AF.Silu,
                                                 bias=conv_bt[:, ec:ec + 1])
                            tp_ps = ppt.tile([128, 8, 128], F16, tag="ptr")
                            for tcb in range(8):
                                nc.tensor.transpose(tp_ps[:, tcb, :],
                                                    BT_sb[:, tcb * 128:(tcb + 1) * 128],
                                                    ident16[:])
                            nc.any.tensor_copy(out=B_t[:], in_=tp_ps[:])
                        else:
                            nc.scalar.activation(CT_sb[:], ts[0][:], AF.Silu,
                                                 bias=conv_bt[:, ec:ec + 1])

                    # schedule: 16 z-GEMM slots (2 eqh x 8 tc); sprinkle 18 convs
                    conv_order = [0, 1, 2, 3, 16, 17] + list(range(4, 16))
                    ci = 0
                    while ci < 4:
                        emit_conv(conv_order[ci])
                        ci += 1
                    # ---------- dt/cA machinery ----------
                    with tc.tile_pool(name="pdt", bufs=1) as pdt, \
                         tc.tile_pool(name="pdtps", bufs=1, space="PSUM") as pdtps:
                        dtA = pdt.tile([32, L], F32, tag="dtA")
                        nc.vector.tensor_scalar_mul(dtA[:], dt_sb[:], a_neg[:])
                        lndt = pdt.tile([32, L], F32, tag="lndt")
                        nc.scalar.activation(lndt[:], dt_sb[:], AF.Ln)
                        for t_c in range(TC):
                            sl = slice(t_c * 128, (t_c + 1) * 128)
                            nc.vector.tensor_tensor_scan(cA_row[:, t_c, :], dtA[:, sl], dtA[:, sl],
                                                         initial=0.0, op0=ALU.add, op1=ALU.bypass)
                            pdts = pdtps.tile([128, 32], F32, tag="pq")
                            nc.tensor.transpose(pdts[:], dt_sb[:, sl], ident32[:32, :32])
                            nc.any.tensor_copy(out=dt_T[:, t_c, :], in_=pdts[:])
                            wr = pdt.tile([32, CH], F32, tag="wr")
                            nc.scalar.activation(wr[:], cA_row[:, t_c, :], AF.Exp, scale=-1.0,
                                                 bias=cA_row[:, t_c, 127:128])
                            pdts2 = pdtps.tile([128, 32], F32, tag="pq2")
                            nc.tensor.transpose(pdts2[:], wr[:], ident32[:32, :32])
                            nc.any.tensor_copy(out=w_T[:, t_c, :], in_=pdts2[:])
                            # dtw = dt^T * w^T
                            nc.vector.tensor_tensor(dtw_all[:, t_c, :], dt_T[:, t_c, :],
                                                    w_T[:, t_c, :], op=ALU.mult)
                            da0_2d = pdt.tile([32, CH], F16, tag="da0_2d")
                            nc.scalar.activation(da0_2d[:], cA_row[:, t_c, :], AF.Exp)
                            nc.gpsimd.dma_start(da0_dram[t_c], da0_2d[:])
                        # --- q/r split of cA and colneg (= -cA + ln dt), row layout ---
                        # q = f16(x + 16384) - 16384 is an exact multiple of 16; r = x - q
                        # is small, so both halves are f16-exact enough for the PE-side
                        # E-argument assembly (PSUM f32 accumulate restores cancellation).
                        cAf = cA_row[:].rearrange("p a b -> p (a b)")
                        cn_row = pdt.tile([32, L], F32, tag="dtA")  # reuse dtA slot
                        nc.vector.scalar_tensor_tensor(cn_row[:], cAf, -1.0, lndt[:],
                                                       op0=ALU.mult, op1=ALU.add)
                        for name, srcap in (("ca", cAf), ("cn", cn_row[:])):
                            qv = pdt.tile([32, L], F16, tag="qv")
                            nc.vector.tensor_copy(out=qv[:], in_=srcap)
                            rv = pdt.tile([32, L], F16, tag="rv")
                            nc.vector.tensor_tensor(rv[:], srcap, qv[:], op=ALU.subtract)
                            qv3 = qv[:].rearrange("p (a b) -> p a b", a=TC)
                            rv3 = rv[:].rearrange("p (a b) -> p a b", a=TC)
                            if name == "ca":
                                nc.gpsimd.dma_start(
                                    caqr_dram[:, 0, :].rearrange("t (h c) -> h t c", h=H), qv3)
                                nc.gpsimd.dma_start(
                                    caqr_dram[:, 1, :].rearrange("t (h c) -> h t c", h=H), rv3)
                            else:
                                nc.gpsimd.dma_start(
                                    cn_dram[:, 0:H, :].rearrange("t k c -> k t c"), qv3)
                                nc.gpsimd.dma_start(
                                    cn_dram[:, H:2 * H, :].rearrange("t k c -> k t c"), rv3)
                        nc.gpsimd.dma_start(cnqr_sb[2:2 + 2 * H],
                                cn_dram.rearrange("t k c -> k t c"))

                    for eqh in range(2):
                        wz = wp3.tile([128, 8, 1024], F16, tag="wz")
                        nc.sync.dma_start(wz[:], w_in_d[:, eqh * 1024:(eqh + 1) * 1024]
                                          .rearrange("(kd p) e -> p kd e", p=128))
                        for t_c in range(8):
                            zp = pz.tile([128, 2, 512], F32, tag="zp")
                            for kd in range(8):
                                for zh in range(2):
                                    nc.tensor.matmul(zp[:, zh, :],
                                                     lhsT=hT[:, kd, t_c * 128:(t_c + 1) * 128],
                                                     rhs=wz[:, kd, zh * 512:(zh + 1) * 512],
                                                     start=(kd == 0), stop=(kd == 7))
                            nc.scalar.activation(
                                sz_sb[:, t_c, eqh * 1024:(eqh + 1) * 1024],
                                zp[:].rearrange("p a b -> p (a b)"), AF.Silu)
                            while ci * 16 < 18 * (eqh * 8 + t_c + 1) and ci < 18:
                                emit_conv(conv_order[ci])
                                ci += 1
                    while ci < 18:
                        emit_conv(conv_order[ci])
                        ci += 1
        # hT pool closed here

        # ---------- Gt precompute: gt_all[tc] = B_c^T C_c ----------
        with tc.tile_pool(name="pgt", bufs=2, space="PSUM") as pgt:
            for t_c in range(TC):
                tsl = slice(t_c * 128, (t_c + 1) * 128)
                gp = pgt.tile([128, CH], F32, tag="gp")
                nc.tensor.matmul(gp[:], lhsT=BT_sb[:, tsl], rhs=CT_sb[:, tsl],
                                 start=True, stop=True)
                nc.scalar.activation(gt_all[:, t_c, :], gp[:], AF.Copy)

        midp_cm.__exit__(None, None, None)

        # ---------- phase 4: scan + gating + rmsnorm + out_proj ----------
        with tc.tile_pool(name="p4r", bufs=1) as p4r, \
             tc.tile_pool(name="p4da", bufs=2) as p4da, \
             tc.tile_pool(name="p4in", bufs=2) as p4in, \
             tc.tile_pool(name="p4ew", bufs=2) as p4ew, \
             tc.tile_pool(name="p4ct", bufs=2) as p4ct, \
             tc.tile_pool(name="p4s", bufs=2) as p4s, \
             tc.tile_pool(name="p4x", bufs=2) as p4x, \
             tc.tile_pool(name="p4g", bufs=2) as p4g, \
             tc.tile_pool(name="psy", bufs=1, space="PSUM") as psy, \
             tc.tile_pool(name="psst", bufs=1, space="PSUM") as psst, \
             tc.tile_pool(name="pearg", bufs=2, space="PSUM") as pearg, \
             tc.tile_pool(name="pstr", bufs=1, space="PSUM") as pstr, \
             tc.tile_pool(name="pso", bufs=1, space="PSUM") as pso:
            # pool budget = pearg2 + psy1 + psst2 + pstr1 + pso2 = 8 banks
            ca_reps, da_reps, x_ins = {}, {}, {}

            def prefetch(t_c):
                if t_c >= TC:
                    return
                nc.sync.dma_start(ind_cas[t_c % 2][0:2, :], caqr_dram[t_c])
                ca_reps[t_c] = ind_cas[t_c % 2]
                da = p4da.tile([128, H, CH], F16, tag="darep")
                nc.sync.dma_start(da[:].rearrange("p a b -> p (a b)"),
                                  da0_dram[t_c:t_c + 1, :].partition_broadcast(128)[:, 0, :])
                da_reps[t_c] = da
                xi = p4in.tile([128, DI], F16, tag="xin")
                nc.sync.dma_start(xi[:], X_dram[t_c])
                x_ins[t_c] = xi

            # chunk-0/1 inputs are small -- issue them ahead of the 4 MB
            # w_out load so the first ea/Ctil don't queue behind it
            prefetch(0)
            dd_sb = p4r.tile([128, H, 128], F16)     # D*I per head
            nc.gpsimd.dma_start(dd_sb[:], d_diag_d.rearrange("h p c -> p h c"))
            w_out_sb = p4r.tile([128, 16, DM], F16)  # [ep, eo, d]  4 MB
            w_out_r = w_out_d.rearrange("(eo p) d -> p eo d", p=128)
            nc.sync.dma_start(w_out_sb[:, 0:8, :], w_out_r[:, 0:8, :])
            nc.gpsimd.dma_start(w_out_sb[:, 8:16, :], w_out_r[:, 8:16, :])
            S_prev = None
            for t_c in range(TC):
                tsl = slice(t_c * 128, (t_c + 1) * 128)
                ca_rep = ca_reps.pop(t_c)
                da0_rep = da_reps.pop(t_c)
                x_in = x_ins.pop(t_c)
                sz_in = sz_sb[:, t_c]
                prefetch(t_c + 1)
                x_sb = x_in[:].rearrange("p (h q) -> p h q", h=H)
                # --- batched E / Mt + per-head Y, pipelined per 8-head quarter:
                # arg assembled on PE (rank-1 ca rows q/r + one-hot colneg rows,
                # f32 PSUM) -> exp -> causal mask -> *Gt -> head matmuls -> gate
                Mt = p4ew.tile([128, H, CH], F16, tag="ew")
                Mt_f = Mt[:].rearrange("p a b -> p (a b)")
                # Ctil (batched): C[t1,n] * da0[h,t1]
                Ctil = p4ct.tile([128, H, CH], F16, tag="Ctil")
                nc.vector.tensor_tensor(Ctil[:],
                                        CT_sb[:, None, tsl].to_broadcast((128, H, CH)),
                                        da0_rep[:], op=ALU.mult)
                # x * dt * w
                xch = p4x.tile([128, H, PH], F16, tag="xchk")
                nc.vector.tensor_tensor(xch[:], x_sb,
                                        dtw_all[:, t_c, :, None].to_broadcast((128, H, PH)),
                                        op=ALU.mult)
                g_sb = p4g.tile([128, DI], F16, tag="g")
                for q4 in range(4):
                    for eh in range(2):
                        o = q4 * 1024 + eh * 512
                        ea = pearg.tile([128, 512], F32, tag="ea")
                        nc.tensor.matmul(ea[:], lhsT=cnqr_sb[:, t_c, :],
                                         rhs=ca_rep[:, o:o + 512], start=True, stop=True)
                        nc.scalar.activation(Mt_f[:, o:o + 512], ea[:], AF.Exp)
                    Mq = Mt[:, q4 * 8:(q4 + 1) * 8, :]
                    nc.gpsimd.affine_select(out=Mq, in_=Mq,
                                            pattern=[[0, 8], [1, CH]],
                                            compare_op=ALU.is_ge, fill=0.0, base=0,
                                            channel_multiplier=-1)
                    nc.vector.tensor_tensor(Mq, Mq,
                                            gt_all[:, t_c, None, :].to_broadcast((128, 8, CH)),
                                            op=ALU.mult)
                    y_ps = psy.tile([128, 8, PH], F32, tag="y")
                    for hh in range(8):
                        h = q4 * 8 + hh
                        nc.tensor.matmul(y_ps[:, hh, :], lhsT=Mt[:, h, :],
                                         rhs=x_sb[:, h, :], start=True, stop=False)
                        if t_c > 0:
                            nc.tensor.matmul(y_ps[:, hh, :], lhsT=Ctil[:, h, :],
                                             rhs=S_prev[:, h, :], start=False, stop=False)
                        nc.tensor.matmul(y_ps[:, hh, :], lhsT=dd_sb[:, h, :],
                                         rhs=x_sb[:, h, :], start=False, stop=True)
                    nc.vector.tensor_tensor(g_sb[:, q4 * 512:(q4 + 1) * 512],
                                            y_ps[:].rearrange("p a b -> p (a b)"),
                                            sz_in[:, q4 * 512:(q4 + 1) * 512],
                                            op=ALU.mult)
                # --- state update: S_new = B^T xch + daend*S_prev (DVE FMA
                # evac; drops the identity matmuls and the Act copy) ---
                S_new = p4s.tile([128, H, PH], F16, tag="S", name="S_new")
                S_new_f = S_new[:].rearrange("p a b -> p (a b)")
                xch_f = xch[:].rearrange("p a b -> p (a b)")
                if t_c > 0:
                    S_dec = p4s.tile([128, H, PH], F16, tag="Sdec", bufs=1)
                    nc.vector.tensor_tensor(S_dec[:], S_prev[:],
                                            da0_rep[:, :, 127:128].to_broadcast((128, H, PH)),
                                            op=ALU.mult)
                    S_dec_f = S_dec[:].rearrange("p a b -> p (a b)")
                for q in range(2):
                    st = psst.tile([128, 2, 512], F32, tag="st")
                    for sh in range(2):
                        o = q * 1024 + sh * 512
                        nc.tensor.matmul(st[:, sh, :], lhsT=B_t[:, t_c, :],
                                         rhs=xch_f[:, o:o + 512],
                                         start=True, stop=True)
                    st_f = st[:].rearrange("p a b -> p (a b)")
                    o2 = q * 1024
                    if t_c > 0:
                        nc.vector.tensor_tensor(S_new_f[:, o2:o2 + 1024], st_f,
                                                S_dec_f[:, o2:o2 + 1024], op=ALU.add)
                    elif q == 0:
                        nc.scalar.activation(S_new_f[:, 0:1024], st_f, AF.Copy)
                    else:
                        nc.vector.tensor_copy(out=S_new_f[:, 1024:2048], in_=st_f)
                S_prev = S_new
                # --- rmsnorm stats (scale applied at out eviction) ---
                gsq = p4g.tile([128, 1024], F32, tag="gsq", bufs=1)
                sq1 = p4g.tile([128, 1], F32, tag="sq1")
                sq2 = p4g.tile([128, 1], F32, tag="sq2")
                nc.scalar.activation(gsq[:], g_sb[:, 0:1024], AF.Square, accum_out=sq1[:])
                nc.scalar.activation(gsq[:], g_sb[:, 1024:2048], AF.Square, accum_out=sq2[:])
                nc.vector.tensor_tensor(sq1[:], sq1[:], sq2[:], op=ALU.add)
                msq = p4g.tile([128, 1], F32, tag="msq")
                nc.vector.tensor_scalar(msq[:], sq1[:], 1.0 / DI, EPS,
                                        op0=ALU.mult, op1=ALU.add)
                rstd = p4g.tile([128, 1], F32, tag="rstd")
                _fast_rsqrt(nc, p4g, rstd[:], msq[:], magic_t[:], (128, 1), "rms")
                # --- transpose g (f16) ---
                yrT = p4g.tile([128, 16, CH], F16, tag="yrT")
                for eg in range(2):
                    tp_ps = pstr.tile([128, 8, 128], F16, tag="ptr4")
                    for j in range(8):
                        eo = eg * 8 + j
                        nc.tensor.transpose(tp_ps[:, j, :], g_sb[:, eo * 128:(eo + 1) * 128],
                                            ident16[:])
                    nc.any.tensor_copy(out=yrT[:, eg * 8:(eg + 1) * 8, :], in_=tp_ps[:])
                # --- out_proj (w_out has norm_w folded in; rstd applied here;
                # eo-outer so each yrT stationary is loaded once for both halves) ---
                po = pso.tile([128, 2, 512], F32, tag="po")
                for eo in range(16):
                    for dh in range(2):
                        nc.tensor.matmul(po[:, dh, :], lhsT=yrT[:, eo, :],
                                         rhs=w_out_sb[:, eo, dh * 512:(dh + 1) * 512],
                                         start=(eo == 0), stop=(eo == 15))
                for dh in range(2):
                    ob = p4g.tile([128, 512], F32, tag="ob")
                    nc.scalar.activation(ob[:], po[:, dh, :], AF.Copy, scale=rstd[:])
                    nc.sync.dma_start(out_d[tsl, dh * 512:(dh + 1) * 512], ob[:])

        if "bt" in dbg_d:
            nc.sync.dma_start(dbg_d["bt"][:], BT_sb[:])
        if "ct" in dbg_d:
            nc.sync.dma_start(dbg_d["ct"][:], CT_sb[:])
        if "dt" in dbg_d:
            nc.sync.dma_start(dbg_d["dt"][:], dt_sb[:])
        if "carow" in dbg_d:
            nc.sync.dma_start(dbg_d["carow"][:], cA_row[:].rearrange("p a b -> p (a b)"))


_NC_CACHE = {}

N_CORES = 8
BSZ = 4


def _get_nc():
    if "nc" not in _NC_CACHE:
        nc = bacc.Bacc("TRN2", target_bir_lowering=False, debug=False,
                       num_devices=N_CORES)
        _NC_CACHE["nc"] = _build(nc)
    return _NC_CACHE["nc"]


def _get_runner():
    """Build the jitted SPMD callable once so repeat kernel() calls skip
    retrace + NEFF recompile (run_bass_via_pjrt builds a fresh closure per
    call, defeating the jit cache)."""
    if "runner" not in _NC_CACHE:
        _NC_CACHE["runner"] = _make_runner(_get_nc())
    return _NC_CACHE["runner"]


def _make_runner(nc):
    import jax
    from jax.sharding import Mesh, PartitionSpec
    from jax.experimental.shard_map import shard_map
    from concourse import bass2jax, mybir as _mb

    bass2jax.install_neuronx_cc_hook()
    partition_name = nc.partition_id_tensor.name if nc.partition_id_tensor else None
    in_names, out_names, out_avals, zero_outs = [], [], [], []
    for alloc in nc.m.functions[0].allocations:
        if not isinstance(alloc, _mb.MemoryLocationSet):
            continue
        name = alloc.memorylocations[0].name
        if alloc.kind == "ExternalInput":
            if name != partition_name:
                in_names.append(name)
        elif alloc.kind == "ExternalOutput":
            shape = tuple(alloc.tensor_shape)
            dtype = _mb.dt.np(alloc.dtype)
            out_names.append(name)
            out_avals.append(jax.core.ShapedArray(shape, dtype))
            zero_outs.append(np.zeros(shape, dtype))
    n_params = len(in_names)
    n_outs = len(out_avals)
    all_in_names = list(in_names) + list(out_names)
    if partition_name is not None:
        all_in_names.append(partition_name)
    donate = tuple(range(n_params, n_params + n_outs))

    def _bodyfn(*args):
        operands = list(args)
        if partition_name is not None:
            operands.append(bass2jax.partition_id_tensor())
        outs = bass2jax._bass_exec_p.bind(
            *operands,
            out_avals=tuple(out_avals),
            in_names=tuple(all_in_names),
            out_names=tuple(out_names),
            lowering_input_output_aliases=(),
            sim_require_finite=True,
            sim_require_nnan=True,
            nc=nc,
        )
        return tuple(outs)

    devices = jax.devices()[:N_CORES]
    mesh = Mesh(np.asarray(devices), ("core",))
    in_specs = (PartitionSpec("core"),) * (n_params + n_outs)
    out_specs = (PartitionSpec("core"),) * n_outs
    sharded = jax.jit(
        shard_map(_bodyfn, mesh=mesh, in_specs=in_specs, out_specs=out_specs,
                  check_rep=False),
        donate_argnums=donate, keep_unused=True)

    def run(in_maps):
        per_core = [[np.asarray(m[name]) for name in in_names] for m in in_maps]
        concat_in = [np.concatenate([per_core[c][i] for c in range(N_CORES)], axis=0)
                     for i in range(n_params)]
        concat_zeros = [np.zeros((N_CORES * z.shape[0], *z.shape[1:]), z.dtype)
                        for z in zero_outs]
        out_arrs = sharded(*concat_in, *concat_zeros)
        return [{name: np.asarray(out_arrs[i]).reshape(N_CORES, *out_avals[i].shape)[c]
                 for i, name in enumerate(out_names)}
                for c in range(N_CORES)]

    def make_device_exec(in_maps):
        """For timing: stage inputs on-device once; returns f() that runs one
        execution with on-device zero outputs and blocks until done."""
        from jax.sharding import NamedSharding
        per_core = [[np.asarray(m[name]) for name in in_names] for m in in_maps]
        concat_in = [np.concatenate([per_core[c][i] for c in range(N_CORES)], axis=0)
                     for i in range(n_params)]
        shard = NamedSharding(mesh, PartitionSpec("core"))
        dev_in = [jax.device_put(a, shard) for a in concat_in]
        zero_shapes = [(N_CORES * z.shape[0], *z.shape[1:]) for z in zero_outs]
        zdtypes = [z.dtype for z in zero_outs]
        import jax.numpy as jnp
        mk_zeros = jax.jit(
            lambda: tuple(jnp.zeros(s, d) for s, d in zip(zero_shapes, zdtypes)),
            out_shardings=tuple(shard for _ in zero_shapes))

        def exec_once():
            zs = mk_zeros()
            jax.block_until_ready(zs)
            import time as _t
            t0 = _t.perf_counter()
            outs = sharded(*dev_in, *zs)
            jax.block_until_ready(outs)
            return _t.perf_counter() - t0
        return exec_once

    run.make_device_exec = make_device_exec
    return run


def _smart_flip(X, lengths):
    B, Ln, _ = X.shape
    r = np.arange(Ln)[None, :]
    pos = np.where(r < lengths[:, None], lengths[:, None] - 1 - r, r)
    return np.take_along_axis(X, pos[:, :, None], axis=1)


def _dir_params(in_proj_w, out_proj_w, conv_w, conv_b, dt_bias, A_log, D, norm_w):
    w_in = np.zeros((DM, EPAD), np.float16)
    w_in[:, :EIN] = in_proj_w.T.astype(np.float16)
    ii = np.arange(128)
    d_diag = np.zeros((H, 128, 128), np.float16)
    for h in range(H):
        d_diag[h, ii, ii] = np.float16(D[h])
    e_ind = np.zeros((2 * H, H * 128), np.float16)
    for k in range(2 * H):
        h = k % H
        e_ind[k, h * 128:(h + 1) * 128] = 1.0
    # w_out with norm_w folded in:  out[d] = sum_e yr[e]*rstd * (W[d,e]*normw[e])
    w_out = (out_proj_w * norm_w[None, :]).T
    params = np.zeros((128, 92), np.float32)
    params[:, 0:72] = conv_w.reshape(18, 128, 4).transpose(1, 0, 2).reshape(128, 72)
    params[:, 72:90] = conv_b.reshape(18, 128).T
    params[0:32, 90] = dt_bias
    params[0:32, 91] = (-np.exp(A_log.astype(np.float64))).astype(np.float32)
    return {
        "w_in": w_in,
        "w_out": np.ascontiguousarray(w_out).astype(np.float16),
        "params": params,
        "d_diag": d_diag,
        "e_ind": e_ind,
    }


def kernel(hidden_states, src_key_padding_mask, in_proj_w, out_proj_w,
           conv_w_f, conv_b_f, dt_bias_f, A_log_f, D_f, norm_w_f,
           conv_w_r, conv_b_r, dt_bias_r, A_log_r, D_r, norm_w_r):
    hidden_states = np.asarray(hidden_states, np.float32)
    mask = np.asarray(src_key_padding_mask)
    lengths = (~mask).sum(axis=1)
    rev = _smart_flip(hidden_states, lengths)

    pf = _dir_params(np.asarray(in_proj_w), np.asarray(out_proj_w),
                     np.asarray(conv_w_f), np.asarray(conv_b_f),
                     np.asarray(dt_bias_f), np.asarray(A_log_f),
                     np.asarray(D_f), np.asarray(norm_w_f))
    pr = _dir_params(np.asarray(in_proj_w), np.asarray(out_proj_w),
                     np.asarray(conv_w_r), np.asarray(conv_b_r),
                     np.asarray(dt_bias_r), np.asarray(A_log_r),
                     np.asarray(D_r), np.asarray(norm_w_r))

    run = _get_runner()
    in_maps = []
    for core in range(N_CORES):
        d, b = divmod(core, BSZ)
        u = hidden_states[b] if d == 0 else rev[b]
        m = dict(pf if d == 0 else pr)
        m["u"] = np.ascontiguousarray(u)
        in_maps.append(m)
    results = run(in_maps)
    out_f = np.stack([results[b]["out"] for b in range(BSZ)])
    out_r = np.stack([results[BSZ + b]["out"] for b in range(BSZ)])
    out_r = _smart_flip(out_r, lengths)
    out = (out_f.astype(np.float64) + out_r.astype(np.float64)) / 2.0
    mu = out.mean(-1, keepdims=True)
    v = out.var(-1, keepdims=True)
    out = (out - mu) / np.sqrt(v + EPS)
    return out.astype(np.float32)



# revision 44
# speedup vs baseline: 1.0027x; 1.0027x over previous
"""BiMamba (bidirectional Mamba2) Trainium2 kernel.

Sharding: 8 NeuronCores = 2 directions x 4 batch sequences; each core runs
the full Mamba2 block (LN -> in_proj -> conv -> chunked SSM scan -> gated
RMSNorm -> out_proj) for one (direction, batch) pair. Host does the
(cheap) sequence flip for the reverse direction and the final
average + LayerNorm combine.

v2: engine-rebalanced. Phase 4 builds the per-chunk decay matrices with
batched ops (one ACT exp per chunk instead of 32; DVE f16 2x-mode adds),
conv runs on DVE (tensor_scalar 4x mode) overlapped with the z GEMM on PE,
sz stays resident in SBUF, the SSM state is kept in f16 with the
decayed-state add done via an identity matmul into PSUM, norm_w is folded
into w_out on the host, and rstd is applied at out_proj eviction.
"""
import numpy as np
import concourse.bass as bass
import concourse.tile as tile
from concourse import bacc, mybir
from concourse import bass_utils
from concourse.masks import make_identity

F32 = mybir.dt.float32
F16 = mybir.dt.float16
I32 = mybir.dt.int32
AF = mybir.ActivationFunctionType
ALU = mybir.AluOpType
AX = mybir.AxisListType

L = 1024          # seq len
DM = 1024         # d_model
DI = 2048         # d_inner
H = 32            # nheads
PH = 64           # headdim
NS = 128          # d_state
CONV = 2304       # conv channels
EIN = 4384        # in_proj out dim
EPAD = 4480       # padded (35*128)
TC = 8            # time chunks
CH = 128          # chunk length
EPS = 1e-5
NEG = -30000.0


def _fast_rsqrt(nc, pool, out_ap, x_ap, magic_bcast, shape, tag):
    """out = 1/sqrt(x) via int bit-hack + 2 Newton iterations (DVE only).
    x_ap must be positive. shape = (128, n). magic_bcast: int32 AP broadcast
    of 0x5f3759df matching shape."""
    n = shape[1]
    sh = pool.tile([128, n], I32, tag=tag + "_sh")
    nc.vector.tensor_scalar(sh[:], x_ap.bitcast(I32), 1, None,
                            op0=ALU.logical_shift_right)
    y = pool.tile([128, n], F32, tag=tag + "_y")
    nc.vector.scalar_tensor_tensor(y[:].bitcast(I32), magic_bcast, 0,
                                   sh[:], op0=ALU.bypass, op1=ALU.subtract)
    xh = pool.tile([128, n], F32, tag=tag + "_xh")
    nc.vector.tensor_scalar_mul(xh[:], x_ap, 0.5)
    t = pool.tile([128, n], F32, tag=tag + "_t")
    for _ in range(2):
        nc.vector.tensor_tensor(t[:], y[:], y[:], op=ALU.mult)
        nc.vector.tensor_tensor(t[:], t[:], xh[:], op=ALU.mult)
        nc.vector.tensor_scalar(t[:], t[:], -1.0, 1.5, op0=ALU.mult, op1=ALU.add)
        nc.vector.tensor_tensor(y[:], y[:], t[:], op=ALU.mult)
    nc.vector.tensor_copy(out=out_ap, in_=y[:])


def _declare(nc):
    u_d = nc.dram_tensor("u", [L, DM], F32, kind="ExternalInput").ap()
    w_in_d = nc.dram_tensor("w_in", [DM, EPAD], F16, kind="ExternalInput").ap()
    w_out_d = nc.dram_tensor("w_out", [DI, DM], F16, kind="ExternalInput").ap()
    # packed small params: cols 0:72 conv_wt, 72:90 conv_bt,
    # 90 dt_bias (rows 0:32), 91 a_neg (rows 0:32)
    params_d = nc.dram_tensor("params", [128, 92], F32, kind="ExternalInput").ap()
    d_diag_d = nc.dram_tensor("d_diag", [H, 128, 128], F16, kind="ExternalInput").ap()
    e_ind_d = nc.dram_tensor("e_ind", [2 * H, H * CH], F16, kind="ExternalInput").ap()
    out_d = nc.dram_tensor("out", [L, DM], F32, kind="ExternalOutput").ap()
    return (u_d, w_in_d, w_out_d, params_d, d_diag_d, e_ind_d, out_d)


def _build(nc, repeats=1):
    args = _declare(nc)
    with tile.TileContext(nc) as tc:
        for _ in range(repeats):
            _body(nc, tc, *args, {})
    nc.compile()
    return nc


def _body(nc, tc, u_d, w_in_d, w_out_d, params_d, d_diag_d, e_ind_d, out_d, dbg_d):
    from contextlib import ExitStack
    ctx = ExitStack()
    with ctx:
        # ---------- constants / small params (whole-kernel lifetime) ----------
        const_p = ctx.enter_context(tc.tile_pool(name="const", bufs=1))
        ident16 = const_p.tile([128, 128], F16)
        make_identity(nc, ident16)
        ident32 = const_p.tile([128, 128], F32)
        make_identity(nc, ident32)
        maskB = const_p.tile([128, 128], F32)
        nc.gpsimd.memset(maskB[:], 0.0)
        # keep where t1 (free) >= t2 (partition): iota = free - part >= 0
        nc.gpsimd.affine_select(out=maskB[:], in_=maskB[:], pattern=[[1, 128]],
                                compare_op=ALU.is_ge, fill=NEG, base=0,
                                channel_multiplier=-1)
        eps_t = const_p.tile([128, 1], F32)
        nc.gpsimd.memset(eps_t[:], EPS)
        magic_t = const_p.tile([128, 1], I32)
        nc.gpsimd.memset(magic_t[:], 0x5F3759DF)
        params = const_p.tile([128, 92], F32)
        nc.sync.dma_start(params[:], params_d[:])
        conv_wt = params[:, 0:72]
        conv_bt = params[:, 72:90]
        dt_bias = params[0:32, 90:91]
        a_neg = params[0:32, 91:92]
        ones2 = const_p.tile([2, 128], F16)
        nc.gpsimd.memset(ones2[:], 1.0)

        # ---------- mid-size residents ----------
        res_p = ctx.enter_context(tc.tile_pool(name="res", bufs=1))
        BT_sb = res_p.tile([128, L], F16)         # [n, t]
        CT_sb = res_p.tile([128, L], F16)         # [n, t]
        B_t = res_p.tile([128, TC, NS], F16)      # [tp, tc, n]
        cnqr_sb = res_p.tile([2 + 2 * H, TC, CH], F16)  # [ones;colneg q/r] rows
        nc.gpsimd.memset(cnqr_sb[0:2, :, :], 1.0)
        ind_cas = []
        for par in range(2):
            t = res_p.tile([2 + 2 * H, H * CH], F16, name=f"indca{par}")
            ind_cas.append(t)
        nc.gpsimd.dma_start(ind_cas[0][2:, :], e_ind_d[:])
        nc.gpsimd.memset(ind_cas[0][0:2, :], 0.0)
        nc.vector.tensor_copy(out=ind_cas[1][:, :], in_=ind_cas[0][:, :])
        dtw_all = res_p.tile([128, TC, H], F16)   # dt * w per chunk
        gt_all = res_p.tile([128, TC, CH], F16)   # B^T C per chunk
        sz_sb = res_p.tile([128, TC, DI], F16)    # silu(z), resident  4 MB
        midp_cm = tc.tile_pool(name="midp", bufs=1)
        midp = midp_cm.__enter__()
        dt_sb = midp.tile([32, L], F32)           # [h, t]
        dt_T = midp.tile([128, TC, H], F16)       # [tp, tc, h]
        w_T = midp.tile([128, TC, H], F16)        # decay-to-chunk-end
        cA_row = midp.tile([32, TC, CH], F32)     # [h, tc, t]

        _uid = nc.next_id()
        caqr_dram = nc.dram_tensor(f"caqr_{_uid}", [TC, 2, H * CH], F16).ap()
        cn_dram = nc.dram_tensor(f"cn_{_uid}", [TC, 2 * H, CH], F16).ap()
        da0_dram = nc.dram_tensor(f"da0_bcast_{_uid}", [TC, H * CH], F16).ap()
        X_dram = nc.dram_tensor(f"x_spill_{_uid}", [TC, 128, DI], F16).ap()

        with tc.tile_pool(name="hTp", bufs=1) as hTp:
            hT = hTp.tile([128, 8, L], F16)        # [dp, do, t]   2 MB
            # ---------- phase 1: LN(u) -> h, transpose -> hT ----------
            with tc.tile_pool(name="ph1", bufs=2) as p1, \
                 tc.tile_pool(name="ph1u", bufs=8) as p1u, \
                 tc.tile_pool(name="ph1s", bufs=2) as p1s, \
                 tc.tile_pool(name="ph1ps", bufs=4, space="PSUM") as p1ps:
                dma_engs = [nc.sync, nc.gpsimd]
                for half in range(2):
                    u_ts = []
                    ssum_all = p1s.tile([128, 4], F32, tag="ssum")
                    ssq_all = p1s.tile([128, 4], F32, tag="ssq")
                    for j in range(4):
                        t_c = half * 4 + j
                        u_t = p1u.tile([128, DM], F32, tag="u", name=f"u{t_c}")
                        dma_engs[t_c % 2].dma_start(u_t[:], u_d[t_c * 128:(t_c + 1) * 128, :])
                        u_ts.append(u_t)
                        nc.vector.tensor_reduce(ssum_all[:, j:j + 1], u_t[:],
                                                axis=AX.X, op=ALU.add)
                        sq = p1.tile([128, DM], F32, tag="sq")
                        nc.scalar.activation(sq[:], u_t[:], AF.Square,
                                             accum_out=ssq_all[:, j:j + 1])
                    nmean = p1s.tile([128, 4], F32, tag="nmean")
                    nc.vector.tensor_scalar_mul(nmean[:], ssum_all[:], -1.0 / DM)
                    var = p1s.tile([128, 4], F32, tag="var")
                    nc.vector.tensor_tensor(var[:], nmean[:], nmean[:], op=ALU.mult)
                    nc.vector.scalar_tensor_tensor(var[:], ssq_all[:], 1.0 / DM, var[:],
                                                   op0=ALU.mult, op1=ALU.subtract)
                    nc.vector.tensor_scalar_add(var[:], var[:], EPS)
                    rstd = p1s.tile([128, 4], F32, tag="rstd")
                    _fast_rsqrt(nc, p1s, rstd[:], var[:],
                                magic_t[:].to_broadcast((128, 4)), (128, 4), "ln")
                    bias2 = p1s.tile([128, 4], F32, tag="bias2")
                    nc.vector.tensor_tensor(bias2[:], nmean[:], rstd[:], op=ALU.mult)
                    for j in range(4):
                        t_c = half * 4 + j
                        h_t = p1.tile([128, DM], F16, tag="h")
                        if t_c % 2 == 0:
                            nc.scalar.activation(h_t[:], u_ts[j][:], AF.Identity,
                                                 bias=bias2[:, j:j + 1],
                                                 scale=rstd[:, j:j + 1])
                        else:
                            nc.vector.tensor_scalar(h_t[:], u_ts[j][:],
                                                    rstd[:, j:j + 1],
                                                    bias2[:, j:j + 1],
                                                    op0=ALU.mult, op1=ALU.add)
                        for dq in range(2):
                            ps = p1ps.tile([128, 4, 128], F16, tag="trq")
                            for jj in range(4):
                                do = dq * 4 + jj
                                nc.tensor.transpose(ps[:, jj, :],
                                                    h_t[:, do * 128:(do + 1) * 128],
                                                    ident16[:])
                            nc.any.tensor_copy(
                                out=hT[:, dq * 4:(dq + 1) * 4, t_c * 128:(t_c + 1) * 128],
                                in_=ps[:])
            if "hT" in dbg_d:
                for a in range(8):
                    nc.sync.dma_start(dbg_d["hT"][a * 128:(a + 1) * 128, :], hT[:, a, :])

            # ---------- phase 2a: in_proj xBC/dt GEMM -> raw xr staging ----------
            with tc.tile_pool(name="p2xr", bufs=1) as xrp:
                xr_all = xrp.tile([128, 18, 1030], F16)    # 3 zero-pad + 1024 + align
                nc.gpsimd.memset(xr_all[:, :, 0:3], 0.0)
                with tc.tile_pool(name="p2w", bufs=4) as wp, \
                     tc.tile_pool(name="p2ps", bufs=2, space="PSUM") as pps:
                    # dt block (ec=18) first: its softplus + the dt/cA
                    # machinery (Exp/Ln, act set 6) then run during the
                    # Copy-only GEMM-evac stream instead of interleaving with
                    # the conv Silu ops (act set 18) -- far fewer table swaps
                    for ec in [18] + list(range(18)):
                        e0 = DI + ec * 128
                        m = 128 if ec < 18 else 32
                        ps = pps.tile([128, 2, 512], F32, tag="px")
                        wt = wp.tile([128, 8, 128], F16, tag="w")
                        nc.sync.dma_start(wt[:], w_in_d[:, e0:e0 + 128]
                                          .rearrange("(kd p) e -> p kd e", p=128))
                        for kd in range(8):
                            for th in range(2):
                                nc.tensor.matmul(ps[:m, th], lhsT=wt[:, kd, :m],
                                                 rhs=hT[:, kd, th * 512:(th + 1) * 512],
                                                 start=(kd == 0), stop=(kd == 7))
                        if ec == 18:
                            # softplus(x + dt_bias) = ln(1 + exp(x + dt_bias))
                            nc.scalar.activation(dt_sb[:], ps[:32].rearrange("p a b -> p (a b)"),
                                                 AF.Exp, bias=dt_bias[:])
                            nc.scalar.activation(dt_sb[:], dt_sb[:], AF.Ln, bias=1.0)
                        else:
                            nc.scalar.activation(xr_all[:, ec, 3:3 + L],
                                                 ps[:].rearrange("p a b -> p (a b)"),
                                                 AF.Copy)

                # ------- phase 2b + 3 interleaved: conv (DVE) + z GEMM (PE) -------
                # conv for one ec: 4 tensor_scalar (4x) + add tree -> silu -> out
                with tc.tile_pool(name="p3w", bufs=1) as wp3, \
                     tc.tile_pool(name="p3cv", bufs=3) as cvp, \
                     tc.tile_pool(name="p3ps", bufs=2, space="PSUM") as pz, \
                     tc.tile_pool(name="p3pt", bufs=2, space="PSUM") as ppt:

                    def emit_conv(ec):
                        xr = xr_all[:, ec, :]
                        ts = []
                        for k in range(4):
                            t = cvp.tile([128, L], F16, tag=f"cv{k}")
                            nc.vector.tensor_scalar(
                                t[:], xr[:, k:k + L], conv_wt[:, ec * 4 + k:ec * 4 + k + 1],
                                None, op0=ALU.mult)
                            ts.append(t)
                        nc.vector.tensor_tensor(ts[0][:], ts[0][:], ts[1][:], op=ALU.add)
                        nc.vector.tensor_tensor(ts[2][:], ts[2][:], ts[3][:], op=ALU.add)
                        nc.vector.tensor_tensor(ts[0][:], ts[0][:], ts[2][:], op=ALU.add)
                        if ec <= 15:
                            xa = cvp.tile([128, L], F16, tag="xact")
                            nc.scalar.activation(xa[:], ts[0][:], AF.Silu,
                                                 bias=conv_bt[:, ec:ec + 1])
                            tp_ps = ppt.tile([128, 8, 128], F16, tag="ptr")
                            for tcb in range(8):
                                nc.tensor.transpose(tp_ps[:, tcb, :],
                                                    xa[:, tcb * 128:(tcb + 1) * 128],
                                                    ident16[:])
                            xts = cvp.tile([128, 8, 128], F16, tag="xts")
                            nc.any.tensor_copy(out=xts[:], in_=tp_ps[:])
                            nc.sync.dma_start(
                                X_dram[:, :, ec * 128:(ec + 1) * 128]
                                .rearrange("a p c -> p a c"), xts[:])
                        elif ec == 16:
                            nc.scalar.activation(BT_sb[:], ts[0][:], AF.Silu,
                                                 bias=conv_bt[:, ec:ec + 1])
                            tp_ps = ppt.tile([128, 8, 128], F16, tag="ptr")
                            for tcb in range(8):
                                nc.tensor.transpose(tp_ps[:, tcb, :],
                                                    BT_sb[:, tcb * 128:(tcb + 1) * 128],
                                                    ident16[:])
                            nc.any.tensor_copy(out=B_t[:], in_=tp_ps[:])
                        else:
                            nc.scalar.activation(CT_sb[:], ts[0][:], AF.Silu,
                                                 bias=conv_bt[:, ec:ec + 1])

                    # schedule: 16 z-GEMM slots (2 eqh x 8 tc); sprinkle 18 convs
                    conv_order = [0, 1, 2, 3, 16, 17] + list(range(4, 16))
                    ci = 0
                    while ci < 4:
                        emit_conv(conv_order[ci])
                        ci += 1
                    # ---------- dt/cA machinery ----------
                    with tc.tile_pool(name="pdt", bufs=1) as pdt, \
                         tc.tile_pool(name="pdtps", bufs=1, space="PSUM") as pdtps:
                        dtA = pdt.tile([32, L], F32, tag="dtA")
                        nc.vector.tensor_scalar_mul(dtA[:], dt_sb[:], a_neg[:])
                        lndt = pdt.tile([32, L], F32, tag="lndt")
                        nc.scalar.activation(lndt[:], dt_sb[:], AF.Ln)
                        for t_c in range(TC):
                            sl = slice(t_c * 128, (t_c + 1) * 128)
                            nc.vector.tensor_tensor_scan(cA_row[:, t_c, :], dtA[:, sl], dtA[:, sl],
                                                         initial=0.0, op0=ALU.add, op1=ALU.bypass)
                            pdts = pdtps.tile([128, 32], F32, tag="pq")
                            nc.tensor.transpose(pdts[:], dt_sb[:, sl], ident32[:32, :32])
                            nc.any.tensor_copy(out=dt_T[:, t_c, :], in_=pdts[:])
                            wr = pdt.tile([32, CH], F32, tag="wr")
                            nc.scalar.activation(wr[:], cA_row[:, t_c, :], AF.Exp, scale=-1.0,
                                                 bias=cA_row[:, t_c, 127:128])
                            pdts2 = pdtps.tile([128, 32], F32, tag="pq2")
                            nc.tensor.transpose(pdts2[:], wr[:], ident32[:32, :32])
                            nc.any.tensor_copy(out=w_T[:, t_c, :], in_=pdts2[:])
                            # dtw = dt^T * w^T
                            nc.vector.tensor_tensor(dtw_all[:, t_c, :], dt_T[:, t_c, :],
                                                    w_T[:, t_c, :], op=ALU.mult)
                            da0_2d = pdt.tile([32, CH], F16, tag="da0_2d")
                            nc.scalar.activation(da0_2d[:], cA_row[:, t_c, :], AF.Exp)
                            nc.gpsimd.dma_start(da0_dram[t_c], da0_2d[:])
                        # --- q/r split of cA and colneg (= -cA + ln dt), row layout ---
                        # q = f16(x + 16384) - 16384 is an exact multiple of 16; r = x - q
                        # is small, so both halves are f16-exact enough for the PE-side
                        # E-argument assembly (PSUM f32 accumulate restores cancellation).
                        cAf = cA_row[:].rearrange("p a b -> p (a b)")
                        cn_row = pdt.tile([32, L], F32, tag="dtA")  # reuse dtA slot
                        nc.vector.scalar_tensor_tensor(cn_row[:], cAf, -1.0, lndt[:],
                                                       op0=ALU.mult, op1=ALU.add)
                        for name, srcap in (("ca", cAf), ("cn", cn_row[:])):
                            qv = pdt.tile([32, L], F16, tag="qv")
                            nc.vector.tensor_copy(out=qv[:], in_=srcap)
                            rv = pdt.tile([32, L], F16, tag="rv")
                            nc.vector.tensor_tensor(rv[:], srcap, qv[:], op=ALU.subtract)
                            qv3 = qv[:].rearrange("p (a b) -> p a b", a=TC)
                            rv3 = rv[:].rearrange("p (a b) -> p a b", a=TC)
                            if name == "ca":
                                nc.gpsimd.dma_start(
                                    caqr_dram[:, 0, :].rearrange("t (h c) -> h t c", h=H), qv3)
                                nc.gpsimd.dma_start(
                                    caqr_dram[:, 1, :].rearrange("t (h c) -> h t c", h=H), rv3)
                            else:
                                nc.gpsimd.dma_start(
                                    cn_dram[:, 0:H, :].rearrange("t k c -> k t c"), qv3)
                                nc.gpsimd.dma_start(
                                    cn_dram[:, H:2 * H, :].rearrange("t k c -> k t c"), rv3)
                        nc.gpsimd.dma_start(cnqr_sb[2:2 + 2 * H],
                                cn_dram.rearrange("t k c -> k t c"))

                    for eqh in range(2):
                        wz = wp3.tile([128, 8, 1024], F16, tag="wz")
                        nc.sync.dma_start(wz[:], w_in_d[:, eqh * 1024:(eqh + 1) * 1024]
                                          .rearrange("(kd p) e -> p kd e", p=128))
                        for t_c in range(8):
                            zp = pz.tile([128, 2, 512], F32, tag="zp")
                            for kd in range(8):
                                for zh in range(2):
                                    nc.tensor.matmul(zp[:, zh, :],
                                                     lhsT=hT[:, kd, t_c * 128:(t_c + 1) * 128],
                                                     rhs=wz[:, kd, zh * 512:(zh + 1) * 512],
                                                     start=(kd == 0), stop=(kd == 7))
                            nc.scalar.activation(
                                sz_sb[:, t_c, eqh * 1024:(eqh + 1) * 1024],
                                zp[:].rearrange("p a b -> p (a b)"), AF.Silu)
                            while ci * 16 < 18 * (eqh * 8 + t_c + 1) and ci < 18:
                                emit_conv(conv_order[ci])
                                ci += 1
                    while ci < 18:
                        emit_conv(conv_order[ci])
                        ci += 1
        # hT pool closed here

        # ---------- Gt precompute: gt_all[tc] = B_c^T C_c ----------
        with tc.tile_pool(name="pgt", bufs=2, space="PSUM") as pgt:
            for t_c in range(TC):
                tsl = slice(t_c * 128, (t_c + 1) * 128)
                gp = pgt.tile([128, CH], F32, tag="gp")
                nc.tensor.matmul(gp[:], lhsT=BT_sb[:, tsl], rhs=CT_sb[:, tsl],
                                 start=True, stop=True)
                nc.scalar.activation(gt_all[:, t_c, :], gp[:], AF.Copy)

        midp_cm.__exit__(None, None, None)

        # ---------- phase 4: scan + gating + rmsnorm + out_proj ----------
        with tc.tile_pool(name="p4r", bufs=1) as p4r, \
             tc.tile_pool(name="p4da", bufs=2) as p4da, \
             tc.tile_pool(name="p4in", bufs=2) as p4in, \
             tc.tile_pool(name="p4ew", bufs=2) as p4ew, \
             tc.tile_pool(name="p4ct", bufs=2) as p4ct, \
             tc.tile_pool(name="p4s", bufs=2) as p4s, \
             tc.tile_pool(name="p4x", bufs=2) as p4x, \
             tc.tile_pool(name="p4g", bufs=2) as p4g, \
             tc.tile_pool(name="psy", bufs=1, space="PSUM") as psy, \
             tc.tile_pool(name="psst", bufs=1, space="PSUM") as psst, \
             tc.tile_pool(name="pearg", bufs=2, space="PSUM") as pearg, \
             tc.tile_pool(name="pstr", bufs=1, space="PSUM") as pstr, \
             tc.tile_pool(name="pso", bufs=1, space="PSUM") as pso:
            # pool budget = pearg2 + psy1 + psst2 + pstr1 + pso2 = 8 banks
            ca_reps, da_reps, x_ins = {}, {}, {}

            def prefetch(t_c):
                if t_c >= TC:
                    return
                nc.sync.dma_start(ind_cas[t_c % 2][0:2, :], caqr_dram[t_c])
                ca_reps[t_c] = ind_cas[t_c % 2]
                da = p4da.tile([128, H, CH], F16, tag="darep")
                nc.sync.dma_start(da[:].rearrange("p a b -> p (a b)"),
                                  da0_dram[t_c:t_c + 1, :].partition_broadcast(128)[:, 0, :])
                da_reps[t_c] = da
                xi = p4in.tile([128, DI], F16, tag="xin")
                nc.sync.dma_start(xi[:], X_dram[t_c])
                x_ins[t_c] = xi

            # chunk-0/1 inputs are small -- issue them ahead of the 4 MB
            # w_out load so the first ea/Ctil don't queue behind it
            prefetch(0)
            dd_sb = p4r.tile([128, H, 128], F16)     # D*I per head
            nc.gpsimd.dma_start(dd_sb[:], d_diag_d.rearrange("h p c -> p h c"))
            w_out_sb = p4r.tile([128, 16, DM], F16)  # [ep, eo, d]  4 MB
            w_out_r = w_out_d.rearrange("(eo p) d -> p eo d", p=128)
            nc.sync.dma_start(w_out_sb[:, 0:8, :], w_out_r[:, 0:8, :])
            nc.gpsimd.dma_start(w_out_sb[:, 8:16, :], w_out_r[:, 8:16, :])
            S_prev = None
            for t_c in range(TC):
                tsl = slice(t_c * 128, (t_c + 1) * 128)
                ca_rep = ca_reps.pop(t_c)
                da0_rep = da_reps.pop(t_c)
                x_in = x_ins.pop(t_c)
                sz_in = sz_sb[:, t_c]
                prefetch(t_c + 1)
                x_sb = x_in[:].rearrange("p (h q) -> p h q", h=H)
                # --- batched E / Mt + per-head Y, pipelined per 8-head quarter:
                # arg assembled on PE (rank-1 ca rows q/r + one-hot colneg rows,
                # f32 PSUM) -> exp -> causal mask -> *Gt -> head matmuls -> gate
                Mt = p4ew.tile([128, H, CH], F16, tag="ew")
                Mt_f = Mt[:].rearrange("p a b -> p (a b)")
                # Ctil (batched): C[t1,n] * da0[h,t1]
                Ctil = p4ct.tile([128, H, CH], F16, tag="Ctil")
                nc.vector.tensor_tensor(Ctil[:],
                                        CT_sb[:, None, tsl].to_broadcast((128, H, CH)),
                                        da0_rep[:], op=ALU.mult)
                # x * dt * w
                xch = p4x.tile([128, H, PH], F16, tag="xchk")
                nc.vector.tensor_tensor(xch[:], x_sb,
                                        dtw_all[:, t_c, :, None].to_broadcast((128, H, PH)),
                                        op=ALU.mult)
                g_sb = p4g.tile([128, DI], F16, tag="g")
                for q4 in range(4):
                    for eh in range(2):
                        o = q4 * 1024 + eh * 512
                        ea = pearg.tile([128, 512], F32, tag="ea")
                        nc.tensor.matmul(ea[:], lhsT=cnqr_sb[:, t_c, :],
                                         rhs=ca_rep[:, o:o + 512], start=True, stop=True)
                        nc.scalar.activation(Mt_f[:, o:o + 512], ea[:], AF.Exp)
                    Mq = Mt[:, q4 * 8:(q4 + 1) * 8, :]
                    nc.gpsimd.affine_select(out=Mq, in_=Mq,
                                            pattern=[[0, 8], [1, CH]],
                                            compare_op=ALU.is_ge, fill=0.0, base=0,
                                            channel_multiplier=-1)
                    nc.vector.tensor_tensor(Mq, Mq,
                                            gt_all[:, t_c, None, :].to_broadcast((128, 8, CH)),
                                            op=ALU.mult)
                    y_ps = psy.tile([128, 8, PH], F32, tag="y")
                    for hh in range(8):
                        h = q4 * 8 + hh
                        nc.tensor.matmul(y_ps[:, hh, :], lhsT=Mt[:, h, :],
                                         rhs=x_sb[:, h, :], start=True, stop=False)
                        if t_c > 0:
                            nc.tensor.matmul(y_ps[:, hh, :], lhsT=Ctil[:, h, :],
                                             rhs=S_prev[:, h, :], start=False, stop=False)
                        nc.tensor.matmul(y_ps[:, hh, :], lhsT=dd_sb[:, h, :],
                                         rhs=x_sb[:, h, :], start=False, stop=True)
                    nc.vector.tensor_tensor(g_sb[:, q4 * 512:(q4 + 1) * 512],
                                            y_ps[:].rearrange("p a b -> p (a b)"),
                                            sz_in[:, q4 * 512:(q4 + 1) * 512],
                                            op=ALU.mult)
                # --- state update: S_new = B^T xch + daend*S_prev (DVE FMA
                # evac; drops the identity matmuls and the Act copy) ---
                S_new = p4s.tile([128, H, PH], F16, tag="S", name="S_new")
                S_new_f = S_new[:].rearrange("p a b -> p (a b)")
                xch_f = xch[:].rearrange("p a b -> p (a b)")
                if t_c > 0:
                    S_dec = p4s.tile([128, H, PH], F16, tag="Sdec", bufs=1)
                    nc.vector.tensor_tensor(S_dec[:], S_prev[:],
                                            da0_rep[:, :, 127:128].to_broadcast((128, H, PH)),
                                            op=ALU.mult)
                    S_dec_f = S_dec[:].rearrange("p a b -> p (a b)")
                for q in range(2):
                    st = psst.tile([128, 2, 512], F32, tag="st")
                    for sh in range(2):
                        o = q * 1024 + sh * 512
                        nc.tensor.matmul(st[:, sh, :], lhsT=B_t[:, t_c, :],
                                         rhs=xch_f[:, o:o + 512],
                                         start=True, stop=True)
                    st_f = st[:].rearrange("p a b -> p (a b)")
                    o2 = q * 1024
                    if t_c > 0:
                        nc.vector.tensor_tensor(S_new_f[:, o2:o2 + 1024], st_f,
                                                S_dec_f[:, o2:o2 + 1024], op=ALU.add)
                    elif q == 0:
                        nc.scalar.activation(S_new_f[:, 0:1024], st_f, AF.Copy)
                    else:
                        nc.vector.tensor_copy(out=S_new_f[:, 1024:2048], in_=st_f)
                S_prev = S_new
                # --- rmsnorm stats (scale applied at out eviction) ---
                gsq = p4g.tile([128, 1024], F32, tag="gsq", bufs=1)
                sq1 = p4g.tile([128, 1], F32, tag="sq1")
                sq2 = p4g.tile([128, 1], F32, tag="sq2")
                nc.scalar.activation(gsq[:], g_sb[:, 0:1024], AF.Square, accum_out=sq1[:])
                nc.scalar.activation(gsq[:], g_sb[:, 1024:2048], AF.Square, accum_out=sq2[:])
                nc.vector.tensor_tensor(sq1[:], sq1[:], sq2[:], op=ALU.add)
                msq = p4g.tile([128, 1], F32, tag="msq")
                nc.vector.tensor_scalar(msq[:], sq1[:], 1.0 / DI, EPS,
                                        op0=ALU.mult, op1=ALU.add)
                rstd = p4g.tile([128, 1], F32, tag="rstd")
                _fast_rsqrt(nc, p4g, rstd[:], msq[:], magic_t[:], (128, 1), "rms")
                # --- transpose g (f16) ---
                yrT = p4g.tile([128, 16, CH], F16, tag="yrT")
                for eg in range(2):
                    tp_ps = pstr.tile([128, 8, 128], F16, tag="ptr4")
                    for j in range(8):
                        eo = eg * 8 + j
                        nc.tensor.transpose(tp_ps[:, j, :], g_sb[:, eo * 128:(eo + 1) * 128],
                                            ident16[:])
                    nc.any.tensor_copy(out=yrT[:, eg * 8:(eg + 1) * 8, :], in_=tp_ps[:])
                # --- out_proj (w_out has norm_w folded in; rstd applied here;
                # eo-outer so each yrT stationary is loaded once for both halves) ---
                po = pso.tile([128, 2, 512], F32, tag="po")
                for eo in range(16):
                    for dh in range(2):
                        nc.tensor.matmul(po[:, dh, :], lhsT=yrT[:, eo, :],
                                         rhs=w_out_sb[:, eo, dh * 512:(dh + 1) * 512],
                                         start=(eo == 0), stop=(eo == 15))
                for dh in range(2):
                    ob = p4g.tile([128, 512], F32, tag="ob")
                    nc.scalar.activation(ob[:], po[:, dh, :], AF.Copy, scale=rstd[:])
                    nc.sync.dma_start(out_d[tsl, dh * 512:(dh + 1) * 512], ob[:])

        if "bt" in dbg_d:
            nc.sync.dma_start(dbg_d["bt"][:], BT_sb[:])
        if "ct" in dbg_d:
            nc.sync.dma_start(dbg_d["ct"][:], CT_sb[:])
        if "dt" in dbg_d:
            nc.sync.dma_start(dbg_d["dt"][:], dt_sb[:])
        if "carow" in dbg_d:
            nc.sync.dma_start(dbg_d["carow"][:], cA_row[:].rearrange("p a b -> p (a b)"))


_NC_CACHE = {}

N_CORES = 8
BSZ = 4


def _get_nc():
    if "nc" not in _NC_CACHE:
        nc = bacc.Bacc("TRN2", target_bir_lowering=False, debug=False,
                       num_devices=N_CORES)
        _NC_CACHE["nc"] = _build(nc)
    return _NC_CACHE["nc"]


def _get_runner():
    """Build the jitted SPMD callable once so repeat kernel() calls skip
    retrace + NEFF recompile (run_bass_via_pjrt builds a fresh closure per
    call, defeating the jit cache)."""
    if "runner" not in _NC_CACHE:
        _NC_CACHE["runner"] = _make_runner(_get_nc())
    return _NC_CACHE["runner"]


def _make_runner(nc):
    import jax
    from jax.sharding import Mesh, PartitionSpec
    from jax.experimental.shard_map import shard_map
    from concourse import bass2jax, mybir as _mb

    bass2jax.install_neuronx_cc_hook()
    partition_name = nc.partition_id_tensor.name if nc.partition_id_tensor else None
    in_names, out_names, out_avals, zero_outs = [], [], [], []
    for alloc in nc.m.functions[0].allocations:
        if not isinstance(alloc, _mb.MemoryLocationSet):
            continue
        name = alloc.memorylocations[0].name
        if alloc.kind == "ExternalInput":
            if name != partition_name:
                in_names.append(name)
        elif alloc.kind == "ExternalOutput":
            shape = tuple(alloc.tensor_shape)
            dtype = _mb.dt.np(alloc.dtype)
            out_names.append(name)
            out_avals.append(jax.core.ShapedArray(shape, dtype))
            zero_outs.append(np.zeros(shape, dtype))
    n_params = len(in_names)
    n_outs = len(out_avals)
    all_in_names = list(in_names) + list(out_names)
    if partition_name is not None:
        all_in_names.append(partition_name)
    donate = tuple(range(n_params, n_params + n_outs))

    def _bodyfn(*args):
        operands = list(args)
        if partition_name is not None:
            operands.append(bass2jax.partition_id_tensor())
        outs = bass2jax._bass_exec_p.bind(
            *operands,
            out_avals=tuple(out_avals),
            in_names=tuple(all_in_names),
            out_names=tuple(out_names),
            lowering_input_output_aliases=(),
            sim_require_finite=True,
            sim_require_nnan=True,
            nc=nc,
        )
        return tuple(outs)

    devices = jax.devices()[:N_CORES]
    mesh = Mesh(np.asarray(devices), ("core",))
    in_specs = (PartitionSpec("core"),) * (n_params + n_outs)
    out_specs = (PartitionSpec("core"),) * n_outs
    sharded = jax.jit(
        shard_map(_bodyfn, mesh=mesh, in_specs=in_specs, out_specs=out_specs,
                  check_rep=False),
        donate_argnums=donate, keep_unused=True)

    def run(in_maps):
        per_core = [[np.asarray(m[name]) for name in in_names] for m in in_maps]
        concat_in = [np.concatenate([per_core[c][i] for c in range(N_CORES)], axis=0)
                     for i in range(n_params)]
        concat_zeros = [np.zeros((N_CORES * z.shape[0], *z.shape[1:]), z.dtype)
                        for z in zero_outs]
        out_arrs = sharded(*concat_in, *concat_zeros)
        return [{name: np.asarray(out_arrs[i]).reshape(N_CORES, *out_avals[i].shape)[c]
                 for i, name in enumerate(out_names)}
                for c in range(N_CORES)]

    def make_device_exec(in_maps):
        """For timing: stage inputs on-device once; returns f() that runs one
        execution with on-device zero outputs and blocks until done."""
        from jax.sharding import NamedSharding
        per_core = [[np.asarray(m[name]) for name in in_names] for m in in_maps]
        concat_in = [np.concatenate([per_core[c][i] for c in range(N_CORES)], axis=0)
                     for i in range(n_params)]
        shard = NamedSharding(mesh, PartitionSpec("core"))
        dev_in = [jax.device_put(a, shard) for a in concat_in]
        zero_shapes = [(N_CORES * z.shape[0], *z.shape[1:]) for z in zero_outs]
        zdtypes = [z.dtype for z in zero_outs]
        import jax.numpy as jnp
        mk_zeros = jax.jit(
            lambda: tuple(jnp.zeros(s, d) for s, d in zip(zero_shapes, zdtypes)),
            out_shardings=tuple(shard for _ in zero_shapes))

        def exec_once():
            zs = mk_zeros()
            jax.block_until_ready(zs)
            import time as _t
            t0 = _t.perf_counter()
            outs = sharded(*dev_in, *zs)
            jax.block_until_ready(outs)
            return _t.perf_counter() - t0
        return exec_once

    run.make_device_exec = make_device_exec
    return run


def _smart_flip(X, lengths):
    B, Ln, _ = X.shape
    r = np.arange(Ln)[None, :]
    pos = np.where(r < lengths[:, None], lengths[:, None] - 1 - r, r)
    return np.take_along_axis(X, pos[:, :, None], axis=1)


def _dir_params(in_proj_w, out_proj_w, conv_w, conv_b, dt_bias, A_log, D, norm_w):
    w_in = np.zeros((DM, EPAD), np.float16)
    w_in[:, :EIN] = in_proj_w.T.astype(np.float16)
    ii = np.arange(128)
    d_diag = np.zeros((H, 128, 128), np.float16)
    for h in range(H):
        d_diag[h, ii, ii] = np.float16(D[h])
    e_ind = np.zeros((2 * H, H * 128), np.float16)
    for k in range(2 * H):
        h = k % H
        e_ind[k, h * 128:(h + 1) * 128] = 1.0
    # w_out with norm_w folded in:  out[d] = sum_e yr[e]*rstd * (W[d,e]*normw[e])
    w_out = (out_proj_w * norm_w[None, :]).T
    params = np.zeros((128, 92), np.float32)
    params[:, 0:72] = conv_w.reshape(18, 128, 4).transpose(1, 0, 2).reshape(128, 72)
    params[:, 72:90] = conv_b.reshape(18, 128).T
    params[0:32, 90] = dt_bias
    params[0:32, 91] = (-np.exp(A_log.astype(np.float64))).astype(np.float32)
    return {
        "w_in": w_in,
        "w_out": np.ascontiguousarray(w_out).astype(np.float16),
        "params": params,
        "d_diag": d_diag,
        "e_ind": e_ind,
    }


def kernel(hidden_states, src_key_padding_mask, in_proj_w, out_proj_w,
           conv_w_f, conv_b_f, dt_bias_f, A_log_f, D_f, norm_w_f,
           conv_w_r, conv_b_r, dt_bias_r, A_log_r, D_r, norm_w_r):
    hidden_states = np.asarray(hidden_states, np.float32)
    mask = np.asarray(src_key_padding_mask)
    lengths = (~mask).sum(axis=1)
    rev = _smart_flip(hidden_states, lengths)

    pf = _dir_params(np.asarray(in_proj_w), np.asarray(out_proj_w),
                     np.asarray(conv_w_f), np.asarray(conv_b_f),
                     np.asarray(dt_bias_f), np.asarray(A_log_f),
                     np.asarray(D_f), np.asarray(norm_w_f))
    pr = _dir_params(np.asarray(in_proj_w), np.asarray(out_proj_w),
                     np.asarray(conv_w_r), np.asarray(conv_b_r),
                     np.asarray(dt_bias_r), np.asarray(A_log_r),
                     np.asarray(D_r), np.asarray(norm_w_r))

    run = _get_runner()
    in_maps = []
    for core in range(N_CORES):
        d, b = divmod(core, BSZ)
        u = hidden_states[b] if d == 0 else rev[b]
        m = dict(pf if d == 0 else pr)
        m["u"] = np.ascontiguousarray(u)
        in_maps.append(m)
    results = run(in_maps)
    out_f = np.stack([results[b]["out"] for b in range(BSZ)])
    out_r = np.stack([results[BSZ + b]["out"] for b in range(BSZ)])
    out_r = _smart_flip(out_r, lengths)
    out = (out_f.astype(np.float64) + out_r.astype(np.float64)) / 2.0
    mu = out.mean(-1, keepdims=True)
    v = out.var(-1, keepdims=True)
    out = (out - mu) / np.sqrt(v + EPS)
    return out.astype(np.float32)



# revision 46
# speedup vs baseline: 1.0275x; 1.0247x over previous
"""BiMamba (bidirectional Mamba2) Trainium2 kernel.

Sharding: 8 NeuronCores = 2 directions x 4 batch sequences; each core runs
the full Mamba2 block (LN -> in_proj -> conv -> chunked SSM scan -> gated
RMSNorm -> out_proj) for one (direction, batch) pair. Host does the
(cheap) sequence flip for the reverse direction and the final
average + LayerNorm combine.

v2: engine-rebalanced. Phase 4 builds the per-chunk decay matrices with
batched ops (one ACT exp per chunk instead of 32; DVE f16 2x-mode adds),
conv runs on DVE (tensor_scalar 4x mode) overlapped with the z GEMM on PE,
sz stays resident in SBUF, the SSM state is kept in f16 with the
decayed-state add done via an identity matmul into PSUM, norm_w is folded
into w_out on the host, and rstd is applied at out_proj eviction.
"""
import numpy as np
import concourse.bass as bass
import concourse.tile as tile
from concourse import bacc, mybir
from concourse import bass_utils
from concourse.masks import make_identity

F32 = mybir.dt.float32
F16 = mybir.dt.float16
I32 = mybir.dt.int32
AF = mybir.ActivationFunctionType
ALU = mybir.AluOpType
AX = mybir.AxisListType

L = 1024          # seq len
DM = 1024         # d_model
DI = 2048         # d_inner
H = 32            # nheads
PH = 64           # headdim
NS = 128          # d_state
CONV = 2304       # conv channels
EIN = 4384        # in_proj out dim
EPAD = 4480       # padded (35*128)
TC = 8            # time chunks
CH = 128          # chunk length
EPS = 1e-5
NEG = -30000.0


def _fast_rsqrt(nc, pool, out_ap, x_ap, magic_bcast, shape, tag):
    """out = 1/sqrt(x) via int bit-hack + 2 Newton iterations (DVE only).
    x_ap must be positive. shape = (128, n). magic_bcast: int32 AP broadcast
    of 0x5f3759df matching shape."""
    n = shape[1]
    sh = pool.tile([128, n], I32, tag=tag + "_sh")
    nc.vector.tensor_scalar(sh[:], x_ap.bitcast(I32), 1, None,
                            op0=ALU.logical_shift_right)
    y = pool.tile([128, n], F32, tag=tag + "_y")
    nc.vector.scalar_tensor_tensor(y[:].bitcast(I32), magic_bcast, 0,
                                   sh[:], op0=ALU.bypass, op1=ALU.subtract)
    xh = pool.tile([128, n], F32, tag=tag + "_xh")
    nc.vector.tensor_scalar_mul(xh[:], x_ap, 0.5)
    t = pool.tile([128, n], F32, tag=tag + "_t")
    for _ in range(2):
        nc.vector.tensor_tensor(t[:], y[:], y[:], op=ALU.mult)
        nc.vector.tensor_tensor(t[:], t[:], xh[:], op=ALU.mult)
        nc.vector.tensor_scalar(t[:], t[:], -1.0, 1.5, op0=ALU.mult, op1=ALU.add)
        nc.vector.tensor_tensor(y[:], y[:], t[:], op=ALU.mult)
    nc.vector.tensor_copy(out=out_ap, in_=y[:])


def _declare(nc):
    u_d = nc.dram_tensor("u", [L, DM], F32, kind="ExternalInput").ap()
    w_in_d = nc.dram_tensor("w_in", [DM, EPAD], F16, kind="ExternalInput").ap()
    w_out_d = nc.dram_tensor("w_out", [DI, DM], F16, kind="ExternalInput").ap()
    # packed small params: cols 0:72 conv_wt, 72:90 conv_bt,
    # 90 dt_bias (rows 0:32), 91 a_neg (rows 0:32)
    params_d = nc.dram_tensor("params", [128, 92], F32, kind="ExternalInput").ap()
    d_diag_d = nc.dram_tensor("d_diag", [H, 128, 128], F16, kind="ExternalInput").ap()
    e_ind_d = nc.dram_tensor("e_ind", [2 * H, H * CH], F16, kind="ExternalInput").ap()
    out_d = nc.dram_tensor("out", [L, DM], F32, kind="ExternalOutput").ap()
    return (u_d, w_in_d, w_out_d, params_d, d_diag_d, e_ind_d, out_d)


def _build(nc, repeats=1):
    args = _declare(nc)
    with tile.TileContext(nc) as tc:
        for _ in range(repeats):
            _body(nc, tc, *args, {})
    nc.compile()
    return nc


def _body(nc, tc, u_d, w_in_d, w_out_d, params_d, d_diag_d, e_ind_d, out_d, dbg_d):
    from contextlib import ExitStack
    ctx = ExitStack()
    with ctx:
        # ---------- constants / small params (whole-kernel lifetime) ----------
        const_p = ctx.enter_context(tc.tile_pool(name="const", bufs=1))
        ident16 = const_p.tile([128, 128], F16)
        make_identity(nc, ident16)
        ident32 = const_p.tile([128, 128], F32)
        make_identity(nc, ident32)
        maskB = const_p.tile([128, 128], F32)
        nc.gpsimd.memset(maskB[:], 0.0)
        # keep where t1 (free) >= t2 (partition): iota = free - part >= 0
        nc.gpsimd.affine_select(out=maskB[:], in_=maskB[:], pattern=[[1, 128]],
                                compare_op=ALU.is_ge, fill=NEG, base=0,
                                channel_multiplier=-1)
        eps_t = const_p.tile([128, 1], F32)
        nc.gpsimd.memset(eps_t[:], EPS)
        magic_t = const_p.tile([128, 1], I32)
        nc.gpsimd.memset(magic_t[:], 0x5F3759DF)
        params = const_p.tile([128, 92], F32)
        nc.sync.dma_start(params[:], params_d[:])
        conv_wt = params[:, 0:72]
        conv_bt = params[:, 72:90]
        dt_bias = params[0:32, 90:91]
        a_neg = params[0:32, 91:92]
        ones2 = const_p.tile([2, 128], F16)
        nc.gpsimd.memset(ones2[:], 1.0)

        # ---------- mid-size residents ----------
        res_p = ctx.enter_context(tc.tile_pool(name="res", bufs=1))
        BT_sb = res_p.tile([128, L], F16)         # [n, t]
        CT_sb = res_p.tile([128, L], F16)         # [n, t]
        B_t = res_p.tile([128, TC, NS], F16)      # [tp, tc, n]
        cnqr_sb = res_p.tile([2 + 2 * H, TC, CH], F16)  # [ones;colneg q/r] rows
        nc.gpsimd.memset(cnqr_sb[0:2, :, :], 1.0)
        ind_cas = []
        for par in range(2):
            t = res_p.tile([2 + 2 * H, H * CH], F16, name=f"indca{par}")
            ind_cas.append(t)
        nc.gpsimd.dma_start(ind_cas[0][2:, :], e_ind_d[:])
        nc.gpsimd.memset(ind_cas[0][0:2, :], 0.0)
        nc.vector.tensor_copy(out=ind_cas[1][:, :], in_=ind_cas[0][:, :])
        dtw_all = res_p.tile([128, TC, H], F16)   # dt * w per chunk
        gt_all = res_p.tile([128, TC, CH], F16)   # B^T C per chunk
        sz_sb = res_p.tile([128, TC, DI], F16)    # silu(z), resident  4 MB
        midp_cm = tc.tile_pool(name="midp", bufs=1)
        midp = midp_cm.__enter__()
        dt_sb = midp.tile([32, L], F32)           # [h, t]
        dt_T = midp.tile([128, TC, H], F16)       # [tp, tc, h]
        w_T = midp.tile([128, TC, H], F16)        # decay-to-chunk-end
        cA_row = midp.tile([32, TC, CH], F32)     # [h, tc, t]

        _uid = nc.next_id()
        caqr_dram = nc.dram_tensor(f"caqr_{_uid}", [TC, 2, H * CH], F16).ap()
        cn_dram = nc.dram_tensor(f"cn_{_uid}", [TC, 2 * H, CH], F16).ap()
        da0_dram = nc.dram_tensor(f"da0_bcast_{_uid}", [TC, H * CH], F16).ap()
        X_dram = nc.dram_tensor(f"x_spill_{_uid}", [TC, 128, DI], F16).ap()

        with tc.tile_pool(name="hTp", bufs=1) as hTp:
            hT = hTp.tile([128, 8, L], F16)        # [dp, do, t]   2 MB
            # ---------- phase 1: LN(u) -> h, transpose -> hT ----------
            with tc.tile_pool(name="ph1", bufs=2) as p1, \
                 tc.tile_pool(name="ph1u", bufs=8) as p1u, \
                 tc.tile_pool(name="ph1s", bufs=2) as p1s, \
                 tc.tile_pool(name="ph1ps", bufs=4, space="PSUM") as p1ps:
                dma_engs = [nc.sync, nc.gpsimd]
                for half in range(2):
                    u_ts = []
                    ssum_all = p1s.tile([128, 4], F32, tag="ssum")
                    ssq_all = p1s.tile([128, 4], F32, tag="ssq")
                    for j in range(4):
                        t_c = half * 4 + j
                        u_t = p1u.tile([128, DM], F32, tag="u", name=f"u{t_c}")
                        dma_engs[t_c % 2].dma_start(u_t[:], u_d[t_c * 128:(t_c + 1) * 128, :])
                        u_ts.append(u_t)
                        nc.vector.tensor_reduce(ssum_all[:, j:j + 1], u_t[:],
                                                axis=AX.X, op=ALU.add)
                        sq = p1.tile([128, DM], F32, tag="sq")
                        nc.scalar.activation(sq[:], u_t[:], AF.Square,
                                             accum_out=ssq_all[:, j:j + 1])
                    nmean = p1s.tile([128, 4], F32, tag="nmean")
                    nc.vector.tensor_scalar_mul(nmean[:], ssum_all[:], -1.0 / DM)
                    var = p1s.tile([128, 4], F32, tag="var")
                    nc.vector.tensor_tensor(var[:], nmean[:], nmean[:], op=ALU.mult)
                    nc.vector.scalar_tensor_tensor(var[:], ssq_all[:], 1.0 / DM, var[:],
                                                   op0=ALU.mult, op1=ALU.subtract)
                    nc.vector.tensor_scalar_add(var[:], var[:], EPS)
                    rstd = p1s.tile([128, 4], F32, tag="rstd")
                    _fast_rsqrt(nc, p1s, rstd[:], var[:],
                                magic_t[:].to_broadcast((128, 4)), (128, 4), "ln")
                    bias2 = p1s.tile([128, 4], F32, tag="bias2")
                    nc.vector.tensor_tensor(bias2[:], nmean[:], rstd[:], op=ALU.mult)
                    for j in range(4):
                        t_c = half * 4 + j
                        h_t = p1.tile([128, DM], F16, tag="h")
                        if t_c % 2 == 0:
                            nc.scalar.activation(h_t[:], u_ts[j][:], AF.Identity,
                                                 bias=bias2[:, j:j + 1],
                                                 scale=rstd[:, j:j + 1])
                        else:
                            nc.vector.tensor_scalar(h_t[:], u_ts[j][:],
                                                    rstd[:, j:j + 1],
                                                    bias2[:, j:j + 1],
                                                    op0=ALU.mult, op1=ALU.add)
                        for dq in range(2):
                            ps = p1ps.tile([128, 4, 128], F16, tag="trq")
                            for jj in range(4):
                                do = dq * 4 + jj
                                nc.tensor.transpose(ps[:, jj, :],
                                                    h_t[:, do * 128:(do + 1) * 128],
                                                    ident16[:])
                            nc.any.tensor_copy(
                                out=hT[:, dq * 4:(dq + 1) * 4, t_c * 128:(t_c + 1) * 128],
                                in_=ps[:])
            if "hT" in dbg_d:
                for a in range(8):
                    nc.sync.dma_start(dbg_d["hT"][a * 128:(a + 1) * 128, :], hT[:, a, :])

            # ---------- phase 2a: in_proj xBC/dt GEMM -> raw xr staging ----------
            with tc.tile_pool(name="p2xr", bufs=1) as xrp:
                xr_all = xrp.tile([128, 18, 1030], F16)    # 3 zero-pad + 1024 + align
                nc.gpsimd.memset(xr_all[:, :, 0:3], 0.0)
                with tc.tile_pool(name="p2w", bufs=4) as wp, \
                     tc.tile_pool(name="p2ps", bufs=2, space="PSUM") as pps:
                    # dt block (ec=18) first: its softplus + the dt/cA
                    # machinery (Exp/Ln, act set 6) then run during the
                    # Copy-only GEMM-evac stream instead of interleaving with
                    # the conv Silu ops (act set 18) -- far fewer table swaps
                    for ec in [18] + list(range(18)):
                        e0 = DI + ec * 128
                        m = 128 if ec < 18 else 32
                        ps = pps.tile([128, 2, 512], F32, tag="px")
                        wt = wp.tile([128, 8, 128], F16, tag="w")
                        nc.sync.dma_start(wt[:], w_in_d[:, e0:e0 + 128]
                                          .rearrange("(kd p) e -> p kd e", p=128))
                        for kd in range(8):
                            for th in range(2):
                                nc.tensor.matmul(ps[:m, th], lhsT=wt[:, kd, :m],
                                                 rhs=hT[:, kd, th * 512:(th + 1) * 512],
                                                 start=(kd == 0), stop=(kd == 7))
                        if ec == 18:
                            # softplus(x + dt_bias) = ln(1 + exp(x + dt_bias))
                            nc.scalar.activation(dt_sb[:], ps[:32].rearrange("p a b -> p (a b)"),
                                                 AF.Exp, bias=dt_bias[:])
                            nc.scalar.activation(dt_sb[:], dt_sb[:], AF.Ln, bias=1.0)
                        else:
                            nc.scalar.activation(xr_all[:, ec, 3:3 + L],
                                                 ps[:].rearrange("p a b -> p (a b)"),
                                                 AF.Copy)

                # ------- phase 2b + 3 interleaved: conv (DVE) + z GEMM (PE) -------
                # conv for one ec: 4 tensor_scalar (4x) + add tree -> silu -> out
                with tc.tile_pool(name="p3w", bufs=1) as wp3, \
                     tc.tile_pool(name="p3cv", bufs=3) as cvp, \
                     tc.tile_pool(name="p3ps", bufs=2, space="PSUM") as pz, \
                     tc.tile_pool(name="p3pt", bufs=2, space="PSUM") as ppt:

                    def emit_conv(ec):
                        xr = xr_all[:, ec, :]
                        ts = []
                        for k in range(4):
                            t = cvp.tile([128, L], F16, tag=f"cv{k}")
                            nc.vector.tensor_scalar(
                                t[:], xr[:, k:k + L], conv_wt[:, ec * 4 + k:ec * 4 + k + 1],
                                None, op0=ALU.mult)
                            ts.append(t)
                        nc.vector.tensor_tensor(ts[0][:], ts[0][:], ts[1][:], op=ALU.add)
                        nc.vector.tensor_tensor(ts[2][:], ts[2][:], ts[3][:], op=ALU.add)
                        nc.vector.tensor_tensor(ts[0][:], ts[0][:], ts[2][:], op=ALU.add)
                        if ec <= 15:
                            xa = cvp.tile([128, L], F16, tag="xact")
                            nc.scalar.activation(xa[:], ts[0][:], AF.Silu,
                                                 bias=conv_bt[:, ec:ec + 1])
                            tp_ps = ppt.tile([128, 8, 128], F16, tag="ptr")
                            for tcb in range(8):
                                nc.tensor.transpose(tp_ps[:, tcb, :],
                                                    xa[:, tcb * 128:(tcb + 1) * 128],
                                                    ident16[:])
                            xts = cvp.tile([128, 8, 128], F16, tag="xts")
                            nc.any.tensor_copy(out=xts[:], in_=tp_ps[:])
                            nc.sync.dma_start(
                                X_dram[:, :, ec * 128:(ec + 1) * 128]
                                .rearrange("a p c -> p a c"), xts[:])
                        elif ec == 16:
                            nc.scalar.activation(BT_sb[:], ts[0][:], AF.Silu,
                                                 bias=conv_bt[:, ec:ec + 1])
                            tp_ps = ppt.tile([128, 8, 128], F16, tag="ptr")
                            for tcb in range(8):
                                nc.tensor.transpose(tp_ps[:, tcb, :],
                                                    BT_sb[:, tcb * 128:(tcb + 1) * 128],
                                                    ident16[:])
                            nc.any.tensor_copy(out=B_t[:], in_=tp_ps[:])
                        else:
                            nc.scalar.activation(CT_sb[:], ts[0][:], AF.Silu,
                                                 bias=conv_bt[:, ec:ec + 1])

                    # schedule: 16 z-GEMM slots (2 eqh x 8 tc); sprinkle 18 convs
                    conv_order = [0, 1, 2, 3, 16, 17] + list(range(4, 16))
                    ci = 0
                    while ci < 4:
                        emit_conv(conv_order[ci])
                        ci += 1
                    # ---------- dt/cA machinery ----------
                    with tc.tile_pool(name="pdt", bufs=1) as pdt, \
                         tc.tile_pool(name="pdtps", bufs=1, space="PSUM") as pdtps:
                        dtA = pdt.tile([32, L], F32, tag="dtA")
                        nc.vector.tensor_scalar_mul(dtA[:], dt_sb[:], a_neg[:])
                        lndt = pdt.tile([32, L], F32, tag="lndt")
                        nc.scalar.activation(lndt[:], dt_sb[:], AF.Ln)
                        for t_c in range(TC):
                            sl = slice(t_c * 128, (t_c + 1) * 128)
                            nc.vector.tensor_tensor_scan(cA_row[:, t_c, :], dtA[:, sl], dtA[:, sl],
                                                         initial=0.0, op0=ALU.add, op1=ALU.bypass)
                            pdts = pdtps.tile([128, 32], F32, tag="pq")
                            nc.tensor.transpose(pdts[:], dt_sb[:, sl], ident32[:32, :32])
                            nc.any.tensor_copy(out=dt_T[:, t_c, :], in_=pdts[:])
                            wr = pdt.tile([32, CH], F32, tag="wr")
                            nc.scalar.activation(wr[:], cA_row[:, t_c, :], AF.Exp, scale=-1.0,
                                                 bias=cA_row[:, t_c, 127:128])
                            pdts2 = pdtps.tile([128, 32], F32, tag="pq2")
                            nc.tensor.transpose(pdts2[:], wr[:], ident32[:32, :32])
                            nc.any.tensor_copy(out=w_T[:, t_c, :], in_=pdts2[:])
                            # dtw = dt^T * w^T
                            nc.vector.tensor_tensor(dtw_all[:, t_c, :], dt_T[:, t_c, :],
                                                    w_T[:, t_c, :], op=ALU.mult)
                            da0_2d = pdt.tile([32, CH], F16, tag="da0_2d")
                            nc.scalar.activation(da0_2d[:], cA_row[:, t_c, :], AF.Exp)
                            nc.gpsimd.dma_start(da0_dram[t_c], da0_2d[:])
                        # --- q/r split of cA and colneg (= -cA + ln dt), row layout ---
                        # q = f16(x + 16384) - 16384 is an exact multiple of 16; r = x - q
                        # is small, so both halves are f16-exact enough for the PE-side
                        # E-argument assembly (PSUM f32 accumulate restores cancellation).
                        cAf = cA_row[:].rearrange("p a b -> p (a b)")
                        cn_row = pdt.tile([32, L], F32, tag="dtA")  # reuse dtA slot
                        nc.vector.scalar_tensor_tensor(cn_row[:], cAf, -1.0, lndt[:],
                                                       op0=ALU.mult, op1=ALU.add)
                        for name, srcap in (("ca", cAf), ("cn", cn_row[:])):
                            qv = pdt.tile([32, L], F16, tag="qv")
                            nc.vector.tensor_copy(out=qv[:], in_=srcap)
                            rv = pdt.tile([32, L], F16, tag="rv")
                            nc.vector.tensor_tensor(rv[:], srcap, qv[:], op=ALU.subtract)
                            qv3 = qv[:].rearrange("p (a b) -> p a b", a=TC)
                            rv3 = rv[:].rearrange("p (a b) -> p a b", a=TC)
                            if name == "ca":
                                nc.gpsimd.dma_start(
                                    caqr_dram[:, 0, :].rearrange("t (h c) -> h t c", h=H), qv3)
                                nc.gpsimd.dma_start(
                                    caqr_dram[:, 1, :].rearrange("t (h c) -> h t c", h=H), rv3)
                            else:
                                nc.gpsimd.dma_start(
                                    cn_dram[:, 0:H, :].rearrange("t k c -> k t c"), qv3)
                                nc.gpsimd.dma_start(
                                    cn_dram[:, H:2 * H, :].rearrange("t k c -> k t c"), rv3)
                        nc.gpsimd.dma_start(cnqr_sb[2:2 + 2 * H],
                                cn_dram.rearrange("t k c -> k t c"))

                    for eqh in range(2):
                        wz = wp3.tile([128, 8, 1024], F16, tag="wz")
                        nc.sync.dma_start(wz[:], w_in_d[:, eqh * 1024:(eqh + 1) * 1024]
                                          .rearrange("(kd p) e -> p kd e", p=128))
                        for t_c in range(8):
                            zp = pz.tile([128, 2, 512], F32, tag="zp")
                            for kd in range(8):
                                for zh in range(2):
                                    nc.tensor.matmul(zp[:, zh, :],
                                                     lhsT=hT[:, kd, t_c * 128:(t_c + 1) * 128],
                                                     rhs=wz[:, kd, zh * 512:(zh + 1) * 512],
                                                     start=(kd == 0), stop=(kd == 7))
                            nc.scalar.activation(
                                sz_sb[:, t_c, eqh * 1024:(eqh + 1) * 1024],
                                zp[:].rearrange("p a b -> p (a b)"), AF.Silu)
                            while ci * 16 < 18 * (eqh * 8 + t_c + 1) and ci < 18:
                                emit_conv(conv_order[ci])
                                ci += 1
                    while ci < 18:
                        emit_conv(conv_order[ci])
                        ci += 1
        # hT pool closed here

        # ---------- Gt precompute: gt_all[tc] = B_c^T C_c ----------
        with tc.tile_pool(name="pgt", bufs=2, space="PSUM") as pgt:
            for t_c in range(TC):
                tsl = slice(t_c * 128, (t_c + 1) * 128)
                gp = pgt.tile([128, CH], F32, tag="gp")
                nc.tensor.matmul(gp[:], lhsT=BT_sb[:, tsl], rhs=CT_sb[:, tsl],
                                 start=True, stop=True)
                nc.scalar.activation(gt_all[:, t_c, :], gp[:], AF.Copy)

        midp_cm.__exit__(None, None, None)

        # ---------- phase 4: scan + gating + rmsnorm + out_proj ----------
        with tc.tile_pool(name="p4r", bufs=1) as p4r, \
             tc.tile_pool(name="p4da", bufs=2) as p4da, \
             tc.tile_pool(name="p4in", bufs=2) as p4in, \
             tc.tile_pool(name="p4ew", bufs=2) as p4ew, \
             tc.tile_pool(name="p4ct", bufs=2) as p4ct, \
             tc.tile_pool(name="p4s", bufs=2) as p4s, \
             tc.tile_pool(name="p4x", bufs=2) as p4x, \
             tc.tile_pool(name="p4g", bufs=2) as p4g, \
             tc.tile_pool(name="psy", bufs=1, space="PSUM") as psy, \
             tc.tile_pool(name="psst", bufs=1, space="PSUM") as psst, \
             tc.tile_pool(name="pearg", bufs=2, space="PSUM") as pearg, \
             tc.tile_pool(name="pstr", bufs=1, space="PSUM") as pstr, \
             tc.tile_pool(name="pso", bufs=1, space="PSUM") as pso:
            # pool budget = pearg2 + psy1 + psst2 + pstr1 + pso2 = 8 banks
            ca_reps, da_reps, x_ins = {}, {}, {}

            def prefetch(t_c):
                if t_c >= TC:
                    return
                nc.sync.dma_start(ind_cas[t_c % 2][0:2, :], caqr_dram[t_c])
                ca_reps[t_c] = ind_cas[t_c % 2]
                da = p4da.tile([128, H, CH], F16, tag="darep")
                nc.sync.dma_start(da[:].rearrange("p a b -> p (a b)"),
                                  da0_dram[t_c:t_c + 1, :].partition_broadcast(128)[:, 0, :])
                da_reps[t_c] = da
                xi = p4in.tile([128, DI], F16, tag="xin")
                nc.sync.dma_start(xi[:], X_dram[t_c])
                x_ins[t_c] = xi

            # chunk-0/1 inputs are small -- issue them ahead of the 4 MB
            # w_out load so the first ea/Ctil don't queue behind it
            prefetch(0)
            dd_sb = p4r.tile([128, H, 128], F16)     # D*I per head
            nc.gpsimd.dma_start(dd_sb[:], d_diag_d.rearrange("h p c -> p h c"))
            w_out_sb = p4r.tile([128, 16, DM], F16)  # [ep, eo, d]  4 MB
            w_out_r = w_out_d.rearrange("(eo p) d -> p eo d", p=128)
            nc.sync.dma_start(w_out_sb[:, 0:8, :], w_out_r[:, 0:8, :])
            nc.gpsimd.dma_start(w_out_sb[:, 8:16, :], w_out_r[:, 8:16, :])
            S_prev = None
            for t_c in range(TC):
                tsl = slice(t_c * 128, (t_c + 1) * 128)
                ca_rep = ca_reps.pop(t_c)
                da0_rep = da_reps.pop(t_c)
                x_in = x_ins.pop(t_c)
                sz_in = sz_sb[:, t_c]
                prefetch(t_c + 1)
                x_sb = x_in[:].rearrange("p (h q) -> p h q", h=H)
                # --- batched E / Mt + per-head Y, pipelined per 8-head quarter:
                # arg assembled on PE (rank-1 ca rows q/r + one-hot colneg rows,
                # f32 PSUM) -> exp -> causal mask -> *Gt -> head matmuls -> gate
                Mt = p4ew.tile([128, H, CH], F16, tag="ew")
                Mt_f = Mt[:].rearrange("p a b -> p (a b)")
                # Ctil (batched): C[t1,n] * da0[h,t1]
                Ctil = p4ct.tile([128, H, CH], F16, tag="Ctil")
                nc.vector.tensor_tensor(Ctil[:],
                                        CT_sb[:, None, tsl].to_broadcast((128, H, CH)),
                                        da0_rep[:], op=ALU.mult)
                # x * dt * w
                xch = p4x.tile([128, H, PH], F16, tag="xchk")
                nc.vector.tensor_tensor(xch[:], x_sb,
                                        dtw_all[:, t_c, :, None].to_broadcast((128, H, PH)),
                                        op=ALU.mult)
                g_sb = p4g.tile([128, DI], F16, tag="g")
                for q4 in range(4):
                    for eh in range(2):
                        o = q4 * 1024 + eh * 512
                        ea = pearg.tile([128, 512], F32, tag="ea")
                        nc.tensor.matmul(ea[:], lhsT=cnqr_sb[:, t_c, :],
                                         rhs=ca_rep[:, o:o + 512], start=True, stop=True)
                        nc.scalar.activation(Mt_f[:, o:o + 512], ea[:], AF.Exp)
                    Mq = Mt[:, q4 * 8:(q4 + 1) * 8, :]
                    nc.gpsimd.affine_select(out=Mq, in_=Mq,
                                            pattern=[[0, 8], [1, CH]],
                                            compare_op=ALU.is_ge, fill=0.0, base=0,
                                            channel_multiplier=-1)
                    nc.vector.tensor_tensor(Mq, Mq,
                                            gt_all[:, t_c, None, :].to_broadcast((128, 8, CH)),
                                            op=ALU.mult)
                    y_ps = psy.tile([128, 8, PH], F32, tag="y")
                    for hh in range(8):
                        h = q4 * 8 + hh
                        nc.tensor.matmul(y_ps[:, hh, :], lhsT=Mt[:, h, :],
                                         rhs=x_sb[:, h, :], start=True, stop=False)
                        if t_c > 0:
                            nc.tensor.matmul(y_ps[:, hh, :], lhsT=Ctil[:, h, :],
                                             rhs=S_prev[:, h, :], start=False, stop=False)
                        nc.tensor.matmul(y_ps[:, hh, :], lhsT=dd_sb[:, h, :],
                                         rhs=x_sb[:, h, :], start=False, stop=True)
                    nc.vector.tensor_tensor(g_sb[:, q4 * 512:(q4 + 1) * 512],
                                            y_ps[:].rearrange("p a b -> p (a b)"),
                                            sz_in[:, q4 * 512:(q4 + 1) * 512],
                                            op=ALU.mult)
                # --- state update: S_new = B^T xch + daend*S_prev (DVE FMA
                # evac; drops the identity matmuls and the Act copy) ---
                S_new = p4s.tile([128, H, PH], F16, tag="S", name="S_new")
                S_new_f = S_new[:].rearrange("p a b -> p (a b)")
                xch_f = xch[:].rearrange("p a b -> p (a b)")
                if t_c > 0:
                    S_dec = p4s.tile([128, H, PH], F16, tag="Sdec", bufs=1)
                    nc.vector.tensor_tensor(S_dec[:], S_prev[:],
                                            da0_rep[:, :, 127:128].to_broadcast((128, H, PH)),
                                            op=ALU.mult)
                    S_dec_f = S_dec[:].rearrange("p a b -> p (a b)")
                for q in range(2):
                    st = psst.tile([128, 2, 512], F32, tag="st")
                    for sh in range(2):
                        o = q * 1024 + sh * 512
                        nc.tensor.matmul(st[:, sh, :], lhsT=B_t[:, t_c, :],
                                         rhs=xch_f[:, o:o + 512],
                                         start=True, stop=True)
                    st_f = st[:].rearrange("p a b -> p (a b)")
                    o2 = q * 1024
                    if t_c > 0:
                        nc.vector.tensor_tensor(S_new_f[:, o2:o2 + 1024], st_f,
                                                S_dec_f[:, o2:o2 + 1024], op=ALU.add)
                    elif q == 0:
                        nc.scalar.activation(S_new_f[:, 0:1024], st_f, AF.Copy)
                    else:
                        nc.vector.tensor_copy(out=S_new_f[:, 1024:2048], in_=st_f)
                S_prev = S_new
                # --- rmsnorm stats (scale applied at out eviction) ---
                gsq = p4g.tile([128, 1024], F32, tag="gsq", bufs=1)
                sq1 = p4g.tile([128, 1], F32, tag="sq1")
                sq2 = p4g.tile([128, 1], F32, tag="sq2")
                nc.scalar.activation(gsq[:], g_sb[:, 0:1024], AF.Square, accum_out=sq1[:])
                nc.scalar.activation(gsq[:], g_sb[:, 1024:2048], AF.Square, accum_out=sq2[:])
                nc.vector.tensor_tensor(sq1[:], sq1[:], sq2[:], op=ALU.add)
                msq = p4g.tile([128, 1], F32, tag="msq")
                nc.vector.tensor_scalar(msq[:], sq1[:], 1.0 / DI, EPS,
                                        op0=ALU.mult, op1=ALU.add)
                rstd = p4g.tile([128, 1], F32, tag="rstd")
                _fast_rsqrt(nc, p4g, rstd[:], msq[:], magic_t[:], (128, 1), "rms")
                # --- transpose g (f16) ---
                yrT = p4g.tile([128, 16, CH], F16, tag="yrT")
                for eg in range(2):
                    tp_ps = pstr.tile([128, 8, 128], F16, tag="ptr4")
                    for j in range(8):
                        eo = eg * 8 + j
                        nc.tensor.transpose(tp_ps[:, j, :], g_sb[:, eo * 128:(eo + 1) * 128],
                                            ident16[:])
                    nc.any.tensor_copy(out=yrT[:, eg * 8:(eg + 1) * 8, :], in_=tp_ps[:])
                # --- out_proj (w_out has norm_w folded in; rstd applied here;
                # eo-outer so each yrT stationary is loaded once for both halves) ---
                po = pso.tile([128, 2, 512], F32, tag="po")
                for eo in range(16):
                    for dh in range(2):
                        nc.tensor.matmul(po[:, dh, :], lhsT=yrT[:, eo, :],
                                         rhs=w_out_sb[:, eo, dh * 512:(dh + 1) * 512],
                                         start=(eo == 0), stop=(eo == 15))
                for dh in range(2):
                    ob = p4g.tile([128, 512], F32, tag="ob")
                    nc.scalar.activation(ob[:], po[:, dh, :], AF.Copy, scale=rstd[:])
                    nc.sync.dma_start(out_d[tsl, dh * 512:(dh + 1) * 512], ob[:])

        if "bt" in dbg_d:
            nc.sync.dma_start(dbg_d["bt"][:], BT_sb[:])
        if "ct" in dbg_d:
            nc.sync.dma_start(dbg_d["ct"][:], CT_sb[:])
        if "dt" in dbg_d:
            nc.sync.dma_start(dbg_d["dt"][:], dt_sb[:])
        if "carow" in dbg_d:
            nc.sync.dma_start(dbg_d["carow"][:], cA_row[:].rearrange("p a b -> p (a b)"))


_NC_CACHE = {}

N_CORES = 8
BSZ = 4


def _get_nc():
    if "nc" not in _NC_CACHE:
        nc = bacc.Bacc("TRN2", target_bir_lowering=False, debug=False,
                       num_devices=N_CORES)
        _NC_CACHE["nc"] = _build(nc)
    return _NC_CACHE["nc"]


def _get_runner():
    """Build the jitted SPMD callable once so repeat kernel() calls skip
    retrace + NEFF recompile (run_bass_via_pjrt builds a fresh closure per
    call, defeating the jit cache)."""
    if "runner" not in _NC_CACHE:
        _NC_CACHE["runner"] = _make_runner(_get_nc())
    return _NC_CACHE["runner"]


def _make_runner(nc):
    import jax
    from jax.sharding import Mesh, PartitionSpec
    from jax.experimental.shard_map import shard_map
    from concourse import bass2jax, mybir as _mb

    bass2jax.install_neuronx_cc_hook()
    partition_name = nc.partition_id_tensor.name if nc.partition_id_tensor else None
    in_names, out_names, out_avals, zero_outs = [], [], [], []
    for alloc in nc.m.functions[0].allocations:
        if not isinstance(alloc, _mb.MemoryLocationSet):
            continue
        name = alloc.memorylocations[0].name
        if alloc.kind == "ExternalInput":
            if name != partition_name:
                in_names.append(name)
        elif alloc.kind == "ExternalOutput":
            shape = tuple(alloc.tensor_shape)
            dtype = _mb.dt.np(alloc.dtype)
            out_names.append(name)
            out_avals.append(jax.core.ShapedArray(shape, dtype))
            zero_outs.append(np.zeros(shape, dtype))
    n_params = len(in_names)
    n_outs = len(out_avals)
    all_in_names = list(in_names) + list(out_names)
    if partition_name is not None:
        all_in_names.append(partition_name)
    donate = tuple(range(n_params, n_params + n_outs))

    def _bodyfn(*args):
        operands = list(args)
        if partition_name is not None:
            operands.append(bass2jax.partition_id_tensor())
        outs = bass2jax._bass_exec_p.bind(
            *operands,
            out_avals=tuple(out_avals),
            in_names=tuple(all_in_names),
            out_names=tuple(out_names),
            lowering_input_output_aliases=(),
            sim_require_finite=True,
            sim_require_nnan=True,
            nc=nc,
        )
        return tuple(outs)

    devices = jax.devices()[:N_CORES]
    mesh = Mesh(np.asarray(devices), ("core",))
    in_specs = (PartitionSpec("core"),) * (n_params + n_outs)
    out_specs = (PartitionSpec("core"),) * n_outs
    sharded = jax.jit(
        shard_map(_bodyfn, mesh=mesh, in_specs=in_specs, out_specs=out_specs,
                  check_rep=False),
        donate_argnums=donate, keep_unused=True)

    def run(in_maps):
        per_core = [[np.asarray(m[name]) for name in in_names] for m in in_maps]
        concat_in = [np.concatenate([per_core[c][i] for c in range(N_CORES)], axis=0)
                     for i in range(n_params)]
        concat_zeros = [np.zeros((N_CORES * z.shape[0], *z.shape[1:]), z.dtype)
                        for z in zero_outs]
        out_arrs = sharded(*concat_in, *concat_zeros)
        return [{name: np.asarray(out_arrs[i]).reshape(N_CORES, *out_avals[i].shape)[c]
                 for i, name in enumerate(out_names)}
                for c in range(N_CORES)]

    def make_device_exec(in_maps):
        """For timing: stage inputs on-device once; returns f() that runs one
        execution with on-device zero outputs and blocks until done."""
        from jax.sharding import NamedSharding
        per_core = [[np.asarray(m[name]) for name in in_names] for m in in_maps]
        concat_in = [np.concatenate([per_core[c][i] for c in range(N_CORES)], axis=0)
                     for i in range(n_params)]
        shard = NamedSharding(mesh, PartitionSpec("core"))
        dev_in = [jax.device_put(a, shard) for a in concat_in]
        zero_shapes = [(N_CORES * z.shape[0], *z.shape[1:]) for z in zero_outs]
        zdtypes = [z.dtype for z in zero_outs]
        import jax.numpy as jnp
        mk_zeros = jax.jit(
            lambda: tuple(jnp.zeros(s, d) for s, d in zip(zero_shapes, zdtypes)),
            out_shardings=tuple(shard for _ in zero_shapes))

        def exec_once():
            zs = mk_zeros()
            jax.block_until_ready(zs)
            import time as _t
            t0 = _t.perf_counter()
            outs = sharded(*dev_in, *zs)
            jax.block_until_ready(outs)
            return _t.perf_counter() - t0
        return exec_once

    run.make_device_exec = make_device_exec
    return run


def _smart_flip(X, lengths):
    B, Ln, _ = X.shape
    r = np.arange(Ln)[None, :]
    pos = np.where(r < lengths[:, None], lengths[:, None] - 1 - r, r)
    return np.take_along_axis(X, pos[:, :, None], axis=1)


def _dir_params(in_proj_w, out_proj_w, conv_w, conv_b, dt_bias, A_log, D, norm_w):
    w_in = np.zeros((DM, EPAD), np.float16)
    w_in[:, :EIN] = in_proj_w.T.astype(np.float16)
    ii = np.arange(128)
    d_diag = np.zeros((H, 128, 128), np.float16)
    for h in range(H):
        d_diag[h, ii, ii] = np.float16(D[h])
    e_ind = np.zeros((2 * H, H * 128), np.float16)
    for k in range(2 * H):
        h = k % H
        e_ind[k, h * 128:(h + 1) * 128] = 1.0
    # w_out with norm_w folded in:  out[d] = sum_e yr[e]*rstd * (W[d,e]*normw[e])
    w_out = (out_proj_w * norm_w[None, :]).T
    params = np.zeros((128, 92), np.float32)
    params[:, 0:72] = conv_w.reshape(18, 128, 4).transpose(1, 0, 2).reshape(128, 72)
    params[:, 72:90] = conv_b.reshape(18, 128).T
    params[0:32, 90] = dt_bias
    params[0:32, 91] = (-np.exp(A_log.astype(np.float64))).astype(np.float32)
    return {
        "w_in": w_in,
        "w_out": np.ascontiguousarray(w_out).astype(np.float16),
        "params": params,
        "d_diag": d_diag,
        "e_ind": e_ind,
    }


def kernel(hidden_states, src_key_padding_mask, in_proj_w, out_proj_w,
           conv_w_f, conv_b_f, dt_bias_f, A_log_f, D_f, norm_w_f,
           conv_w_r, conv_b_r, dt_bias_r, A_log_r, D_r, norm_w_r):
    hidden_states = np.asarray(hidden_states, np.float32)
    mask = np.asarray(src_key_padding_mask)
    lengths = (~mask).sum(axis=1)
    rev = _smart_flip(hidden_states, lengths)

    pf = _dir_params(np.asarray(in_proj_w), np.asarray(out_proj_w),
                     np.asarray(conv_w_f), np.asarray(conv_b_f),
                     np.asarray(dt_bias_f), np.asarray(A_log_f),
                     np.asarray(D_f), np.asarray(norm_w_f))
    pr = _dir_params(np.asarray(in_proj_w), np.asarray(out_proj_w),
                     np.asarray(conv_w_r), np.asarray(conv_b_r),
                     np.asarray(dt_bias_r), np.asarray(A_log_r),
                     np.asarray(D_r), np.asarray(norm_w_r))

    run = _get_runner()
    in_maps = []
    for core in range(N_CORES):
        d, b = divmod(core, BSZ)
        u = hidden_states[b] if d == 0 else rev[b]
        m = dict(pf if d == 0 else pr)
        m["u"] = np.ascontiguousarray(u)
        in_maps.append(m)
    results = run(in_maps)
    out_f = np.stack([results[b]["out"] for b in range(BSZ)])
    out_r = np.stack([results[BSZ + b]["out"] for b in range(BSZ)])
    out_r = _smart_flip(out_r, lengths)
    out = (out_f.astype(np.float64) + out_r.astype(np.float64)) / 2.0
    mu = out.mean(-1, keepdims=True)
    v = out.var(-1, keepdims=True)
    out = (out - mu) / np.sqrt(v + EPS)
    return out.astype(np.float32)

